# revision 1
# baseline (speedup 1.0000x reference)
"""MHSA Trainium2 kernel: B=2, N=2048, H=1024, 16 heads x d=64, fp32.

Sharding: 8 cores = 2 (batch) x 4 (head-groups of 4 heads). Each core is
fully independent (no collectives); host gathers per-core [256, 2048]
transposed outputs into [2, 2048, 1024].

Per-core device plan (all layouts chosen so softmax runs in the
"scores-transposed" orientation: j (keys) on partitions, i (queries) free):
  - inputs: hsT [1024,2048] (host-pretransposed), wqk [1024,512]
    (cols = q0|q1|q2|q3|k0|k1|k2|k3 per-head 64), wv [1024,256], biasj [2048]
    (0 or -30000 additive mask bias).
  - QK projection -> QT/KT per head in [d, tok] layout, duplicated into both
    partition halves so score matmuls can row-tile two j-tiles concurrently
    (contraction d=64 only fills half the PE rows).
  - V projection -> V_aug tiles [tok=128, 4*65] with a ones column per head:
    the attention matmul out = V_aug^T @ P^T (M=65) accumulates the softmax
    denominator in output row 64 for free.
  - scores^T = KT^T @ QT per (head, j-tile), exp via ACT with fused
    scale+mask-bias (per-partition bias = per-key mask), P^T -> SBUF.
  - normalize: reciprocal of l, broadcast across 64 partitions via a K=1
    matmul with a ones vector, multiply, DMA out.
"""

import numpy as np

import concourse.bass as bass
import concourse.bacc as bacc
import concourse.mybir as mybir
import concourse.tile as tile
from concourse.bass_utils import run_bass_kernel_spmd

F32 = mybir.dt.float32
F32R = mybir.dt.float32r
AF = mybir.ActivationFunctionType

HID = 1024
NT = 2048
D = 64
HPC = 4  # heads per core
NCORES = 8
SCALE = float(HID) ** -0.5
KD = HID // 128  # 8 contraction chunks
NJT = NT // 128  # 16 j-tiles
IB = 1024  # i-block
NIB = NT // IB

_CACHE = {}


def _build():
    if "nc" in _CACHE:
        return _CACHE["nc"]
    nc = bacc.Bacc("TRN2", debug=False)
    hsT_d = nc.dram_tensor("hsT", [HID, NT], F32R, kind="ExternalInput")
    wqk_d = nc.dram_tensor("wqk", [HID, 8 * D], F32R, kind="ExternalInput")
    wv_d = nc.dram_tensor("wv", [HID, HPC * D], F32R, kind="ExternalInput")
    bias_d = nc.dram_tensor("biasj", [NT], F32, kind="ExternalInput")
    outT_d = nc.dram_tensor("outT", [HPC * D, NT], F32, kind="ExternalOutput")

    with tile.TileContext(nc) as tc:
        with tc.tile_pool(name="per", bufs=1) as per:
            QTd = [per.tile([128, NT], F32R, tag=f"qtd{h}", name=f"qtd{h}") for h in range(HPC)]
            KTd = [per.tile([128, NT], F32R, tag=f"ktd{h}", name=f"ktd{h}") for h in range(HPC)]
            Vau = [per.tile([128, HPC, 65], F32R, tag=f"vau{t}", name=f"vau{t}") for t in range(NJT)]
            bias_t = per.tile([128, NJT], F32, tag="bias")
            ones64 = per.tile([1, D], F32R, tag="ones")
            nc.vector.memset(ones64[:].bitcast(F32), 1.0)
            nc.sync.dma_start(
                out=bias_t[:], in_=bias_d.ap().rearrange("(a p) -> p a", p=128)
            )
            for t in range(NJT):
                nc.vector.memset(Vau[t][:].bitcast(F32), 1.0)

            with (
                tc.tile_pool(name="ld", bufs=1) as ld,
                tc.tile_pool(name="pp", bufs=1, space="PSUM") as pp,
                tc.tile_pool(name="ppv", bufs=2, space="PSUM") as ppv,
            ):
                hsT = [ld.tile([128, NT], F32R, tag=f"hst{k}", name=f"hst{k}") for k in range(KD)]
                wqk = [ld.tile([128, 8 * D], F32R, tag=f"wqk{k}", name=f"wqk{k}") for k in range(KD)]
                wv = [ld.tile([128, HPC * D], F32R, tag=f"wv{k}", name=f"wv{k}") for k in range(KD)]
                hsT_r = hsT_d.ap().rearrange("(n p) m -> n p m", p=128)
                wqk_r = wqk_d.ap().rearrange("(n p) m -> n p m", p=128)
                wv_r = wv_d.ap().rearrange("(n p) m -> n p m", p=128)
                for k in range(KD):
                    nc.sync.dma_start(out=wqk[k][:], in_=wqk_r[k])
                    nc.sync.dma_start(out=wv[k][:], in_=wv_r[k])
                    nc.sync.dma_start(out=hsT[k][:], in_=hsT_r[k])

                # QK projection. chunk c: 0=[q0|q1] 1=[q2|q3] 2=[k0|k1] 3=[k2|k3]
                for c in range(4):
                    acc = [pp.tile([128, 512], F32, tag=f"pqk{t}", name=f"pqk{c}_{t}") for t in range(4)]
                    for k in range(KD):
                        for t in range(4):
                            nc.tensor.matmul(
                                acc[t][:],
                                wqk[k][:, c * 128 : (c + 1) * 128],
                                hsT[k][:, t * 512 : (t + 1) * 512],
                                start=(k == 0),
                                stop=(k == KD - 1),
                            )
                    dst = QTd if c < 2 else KTd
                    h0 = (c % 2) * 2
                    for t in range(4):
                        nc.vector.tensor_copy(
                            dst[h0][0:64, t * 512 : (t + 1) * 512],
                            acc[t][0:64, :],
                        )
                        nc.vector.tensor_copy(
                            dst[h0 + 1][64:128, t * 512 : (t + 1) * 512],
                            acc[t][64:128, :],
                        )
                # duplicate the filled half into the other partition half
                for h in range(HPC):
                    for dst in (QTd, KTd):
                        if h % 2 == 0:
                            nc.sync.dma_start(
                                out=dst[h][64:128, :], in_=dst[h][0:64, :]
                            )
                        else:
                            nc.sync.dma_start(
                                out=dst[h][0:64, :], in_=dst[h][64:128, :]
                            )

                # V projection: V_aug[t][:, h, 0:64] = v_h rows, col 64 stays 1.0
                for t in range(NJT):
                    pv = ppv.tile([128, HPC * D], F32, tag="pv")
                    for k in range(KD):
                        nc.tensor.matmul(
                            pv[:],
                            hsT[k][:, t * 128 : (t + 1) * 128],
                            wv[k][:],
                            start=(k == 0),
                            stop=(k == KD - 1),
                        )
                    for hh in range(HPC):
                        nc.vector.tensor_copy(
                            Vau[t][:, hh, 0:64], pv[:, hh * D : (hh + 1) * D]
                        )

            # attention
            with (
                tc.tile_pool(name="psc", bufs=3, space="PSUM") as psc,
                tc.tile_pool(name="psv", bufs=1, space="PSUM") as psv,
                tc.tile_pool(name="ptp", bufs=4) as ptp,
                tc.tile_pool(name="stg", bufs=2) as stg,
            ):
                for h in range(HPC):
                    for ib in range(NIB):
                        i0 = ib * IB
                        vout = psv.tile([128, IB], F32, tag="vout")
                        for jtp in range(NJT // 2):
                            jt0, jt1 = 2 * jtp, 2 * jtp + 1
                            sA = psc.tile([128, IB], F32, tag="sc")
                            sB = psc.tile([128, IB], F32, tag="sc")
                            for ic in range(IB // 512):
                                cs = slice(ic * 512, (ic + 1) * 512)
                                qs = slice(i0 + ic * 512, i0 + (ic + 1) * 512)
                                nc.tensor.matmul(
                                    sA[:, cs],
                                    KTd[h][0:64, jt0 * 128 : (jt0 + 1) * 128],
                                    QTd[h][0:64, qs],
                                    start=True,
                                    stop=True,
                                    tile_position=(0, 0),
                                )
                                nc.tensor.matmul(
                                    sB[:, cs],
                                    KTd[h][64:128, jt1 * 128 : (jt1 + 1) * 128],
                                    QTd[h][64:128, qs],
                                    start=True,
                                    stop=True,
                                    tile_position=(64, 0),
                                )
                            ptA = ptp.tile([128, IB], F32R, tag="pt")
                            ptB = ptp.tile([128, IB], F32R, tag="pt")
                            nc.scalar.activation(
                                ptA[:], sA[:], AF.Exp,
                                bias=bias_t[:, jt0 : jt0 + 1], scale=SCALE,
                            )
                            nc.scalar.activation(
                                ptB[:], sB[:], AF.Exp,
                                bias=bias_t[:, jt1 : jt1 + 1], scale=SCALE,
                            )
                            for jt, pt in ((jt0, ptA), (jt1, ptB)):
                                for ic in range(IB // 512):
                                    cs = slice(ic * 512, (ic + 1) * 512)
                                    nc.tensor.matmul(
                                        vout[0:65, cs],
                                        Vau[jt][:, h, :],
                                        pt[:, cs],
                                        start=(jt == 0),
                                        stop=(jt == NJT - 1),
                                    )
                        # normalize: row 64 of vout is l(i)
                        vo = stg.tile([65, IB], F32, tag="vo")
                        nc.vector.tensor_copy(vo[:], vout[0:65, :])
                        rl = stg.tile([1, IB], F32R, tag="rl")
                        with nc.allow_low_precision("f32r is bit-identical to f32"):
                            nc.vector.reciprocal(rl[:], vo[64:65, :])
                        rlb = psc.tile([64, IB], F32, tag="sc")
                        for ic in range(IB // 512):
                            cs = slice(ic * 512, (ic + 1) * 512)
                            nc.tensor.matmul(
                                rlb[:, cs], ones64[:], rl[:, cs],
                                start=True, stop=True,
                            )
                        ot = stg.tile([64, IB], F32, tag="ot")
                        nc.vector.tensor_mul(ot[:], vo[0:64, :], rlb[:])
                        nc.sync.dma_start(
                            out=outT_d.ap()[h * D : (h + 1) * D, i0 : i0 + IB],
                            in_=ot[:],
                        )
    if not nc.is_finalized():
        nc.finalize()
    _CACHE["nc"] = nc
    return nc


def kernel(hidden_states, attention_mask, W_qkv):
    hs = np.asarray(hidden_states, dtype=np.float32)  # [2, 2048, 1024]
    am = np.asarray(attention_mask)  # [2, 2048]
    W = np.asarray(W_qkv, dtype=np.float32)  # [16, 1024, 192]

    nc = _build()
    in_maps = []
    for core in range(NCORES):
        b, hg = core // 4, core % 4
        Wc = W[hg * 4 : hg * 4 + 4]  # [4, 1024, 192]
        q = [Wc[h, :, 0:64] for h in range(4)]
        k = [Wc[h, :, 64:128] for h in range(4)]
        v = [Wc[h, :, 128:192] for h in range(4)]
        in_maps.append(
            {
                "hsT": np.ascontiguousarray(hs[b].T),
                "wqk": np.ascontiguousarray(np.concatenate(q + k, axis=1)),
                "wv": np.ascontiguousarray(np.concatenate(v, axis=1)),
                "biasj": ((am[b] != 0).astype(np.float32) - 1.0) * 30000.0,
            }
        )
    res = run_bass_kernel_spmd(nc, in_maps, list(range(NCORES)))
    if res.exec_time_ns is not None:
        print(f"HW exec time: {res.exec_time_ns} ns")
    if res.mean_exec_time_ns is not None:
        print(f"HW exec time (mean across cores): {res.mean_exec_time_ns} ns")
    out = np.empty((2, NT, HID), dtype=np.float32)
    for core in range(NCORES):
        b, hg = core // 4, core % 4
        out[b, :, hg * 256 : (hg + 1) * 256] = res.results[core]["outT"].T
    return out


def predicted_exec_ns():
    """Device-occupancy estimate for one core (all 8 run the same program
    in parallel). Used by test.py; the real NTFF profiling hook is not
    available in this container."""
    nc = _build()
    from concourse.timeline_sim import TimelineSim
    return float(TimelineSim(nc, trace=False).simulate())



# revision 2
# speedup vs baseline: 1.3906x; 1.3906x over previous
"""MHSA Trainium2 kernel: B=2, N=2048, H=1024, 16 heads x d=64, fp32 in/out.

Sharding: 8 cores = 2 (batch) x 4 (head-groups of 4 heads). Each core is
fully independent (no collectives); host gathers per-core [2048, 256]
outputs into [2, 2048, 1024].

Per-core plan (cost model: matmul = out-free-columns x PE cycle; exp runs
only on the scalar engine at 1 col/cycle/128 partitions, so the kernel is
balanced so PE ~123us and ACT ~133us overlap):
  - All matmul operands bf16 (full-rate at any moving width; fp32 stays in
    PSUM accumulation + the final normalize). Measured rel err ~4e-3.
  - QK proj (W stationary): QT/KT per head in [65, 2048] = [d|aux, tok]
    layout. Row 64 of KT = additive mask bias per key; row 64 of QT = ones,
    so the scores matmul contracts K=65 and adds the mask bias for free
    (scale 1/sqrt(H) is folded into W_q on the host).
  - V proj (tokens stationary): V_aug tiles [tok=128, 4 heads, 65] with a
    ones column per head; the AV matmul's output column 64 then accumulates
    the softmax denominator for free.
  - scores^T[j, i] = KT_aug^T @ QT_aug per (head, j-tile): PSUM [128, 1024].
  - exp via one ACT per (j-tile, i-halfblock): [128, 1024] PSUM -> bf16 P^T
    in SBUF (bias/scale already folded in).
  - AV in the [i, d] orientation: lhsT = P^T chunk [j=128, i=128]
    (stationary), rhs = V_aug [j=128, 65] -> out[i, 65] accumulated over 16
    j-chunks. Halves the AV matmul cost vs the [d, i] orientation and makes
    the denominator a per-partition scalar.
  - normalize: DVE reciprocal of column 64 + tensor_scalar multiply, output
    DMA'd in natural [token, (h d)] layout (no transposes anywhere).
  - Program order pipelines head h+1's QK projection and head h's AV under
    head h's exp stream, which is the critical path.
"""

import numpy as np
from ml_dtypes import bfloat16

import concourse.bass as bass
import concourse.bacc as bacc
import concourse.mybir as mybir
import concourse.tile as tile
from concourse.bass_utils import run_bass_kernel_spmd

F32 = mybir.dt.float32
BF16 = mybir.dt.bfloat16
AF = mybir.ActivationFunctionType

HID = 1024
NT = 2048
D = 64
HPC = 4  # heads per core
NCORES = 8
SCALE = float(HID) ** -0.5
KD = HID // 128  # 8 contraction chunks
NJT = NT // 128  # 16 j-tiles
IB = 1024  # i-block (exp/PSUM unit)
NIB = NT // IB  # 2
NG = IB // 128  # 8 i-groups per i-block

_CACHE = {}


def _build():
    if "nc" in _CACHE:
        return _CACHE["nc"]
    nc = bacc.Bacc("TRN2", debug=False)
    hsT_d = nc.dram_tensor("hsT", [HID, NT], BF16, kind="ExternalInput")
    wqk_d = nc.dram_tensor("wqk", [HID, HPC * 128], BF16, kind="ExternalInput")
    wv_d = nc.dram_tensor("wv", [HID, HPC * D], BF16, kind="ExternalInput")
    aux_d = nc.dram_tensor("aux", [2, NT], BF16, kind="ExternalInput")
    out_d = nc.dram_tensor("out", [NT, HPC * D], F32, kind="ExternalOutput")

    with tile.TileContext(nc) as tc:
        with tc.tile_pool(name="per", bufs=1) as per:
            hsT = [per.tile([128, NT], BF16, tag=f"hst{k}", name=f"hst{k}") for k in range(KD)]
            wqk = [per.tile([128, HPC * 128], BF16, tag=f"wqk{k}", name=f"wqk{k}") for k in range(KD)]
            wv = [per.tile([128, HPC * D], BF16, tag=f"wv{k}", name=f"wv{k}") for k in range(KD)]
            QT = [per.tile([65, NT], BF16, tag=f"qt{h}", name=f"qt{h}") for h in range(HPC)]
            KT = [per.tile([65, NT], BF16, tag=f"kt{h}", name=f"kt{h}") for h in range(HPC)]
            Vau = [per.tile([128, HPC, 65], BF16, tag=f"vau{t}", name=f"vau{t}") for t in range(NJT)]

            hsT_r = hsT_d.ap().rearrange("(n p) m -> n p m", p=128)
            wqk_r = wqk_d.ap().rearrange("(n p) m -> n p m", p=128)
            wv_r = wv_d.ap().rearrange("(n p) m -> n p m", p=128)
            for k in range(KD):
                nc.sync.dma_start(out=wqk[k][:], in_=wqk_r[k])
                nc.sync.dma_start(out=wv[k][:], in_=wv_r[k])
                nc.sync.dma_start(out=hsT[k][:], in_=hsT_r[k])
            for h in range(HPC):
                nc.sync.dma_start(out=KT[h][64:65, :], in_=aux_d.ap()[0:1, :])
                nc.sync.dma_start(out=QT[h][64:65, :], in_=aux_d.ap()[1:2, :])
            for t in range(NJT):
                nc.vector.memset(Vau[t][:, :, 64:65], 1.0)

            with (
                tc.tile_pool(name="psqk", bufs=2, space="PSUM") as psqk,
                tc.tile_pool(name="pacc", bufs=2, space="PSUM") as pacc,
                tc.tile_pool(name="psc", bufs=2, space="PSUM") as psc,
                tc.tile_pool(name="ptp", bufs=2) as ptp,
                tc.tile_pool(name="stg", bufs=2) as stg,
            ):
                # V projection, all heads: V_aug[t][:, h, 0:64] = V rows.
                for t in range(NJT):
                    pv = pacc.tile([128, HPC * D], F32, tag="acc", name="pv")
                    for k in range(KD):
                        nc.tensor.matmul(
                            pv[:],
                            hsT[k][:, t * 128 : (t + 1) * 128],
                            wv[k][:],
                            start=(k == 0),
                            stop=(k == KD - 1),
                        )
                    nc.vector.tensor_copy(
                        Vau[t][:, :, 0:64],
                        pv[:].rearrange("p (h d) -> p h d", h=HPC),
                    )

                def qk_proj(h):
                    # QT/KT rows 0:64 for head h ([q_h | k_h] = 128 W cols).
                    for t in range(4):
                        ts = slice(t * 512, (t + 1) * 512)
                        acc = psqk.tile([128, 512], F32, tag="qk", name="acc")
                        for k in range(KD):
                            nc.tensor.matmul(
                                acc[:],
                                wqk[k][:, h * 128 : (h + 1) * 128],
                                hsT[k][:, ts],
                                start=(k == 0),
                                stop=(k == KD - 1),
                            )
                        nc.vector.tensor_copy(QT[h][0:64, ts], acc[0:64, :])
                        nc.vector.tensor_copy(KT[h][0:64, ts], acc[64:128, :])

                qk_proj(0)

                for h in range(HPC):
                    PTs = [[None] * NJT for _ in range(NIB)]
                    # scores + exp for both i-blocks
                    for ib in range(NIB):
                        i0 = ib * IB
                        for jt in range(NJT):
                            sc = psc.tile([128, IB], F32, tag="sc", name="sc")
                            for ic in range(IB // 512):
                                cs = slice(ic * 512, (ic + 1) * 512)
                                qs = slice(i0 + ic * 512, i0 + (ic + 1) * 512)
                                nc.tensor.matmul(
                                    sc[:, cs],
                                    KT[h][0:65, jt * 128 : (jt + 1) * 128],
                                    QT[h][0:65, qs],
                                    start=True,
                                    stop=True,
                                )
                            pt = ptp.tile([128, IB], BF16, tag=f"pt{jt}", name="pt")
                            nc.scalar.activation(pt[:], sc[:], AF.Exp)
                            PTs[ib][jt] = pt
                    # next head's projection fits under this head's exp stream
                    if h + 1 < HPC:
                        qk_proj(h + 1)
                    # AV + normalize per i-block
                    outsb = stg.tile([128, NT // 128, D], F32, tag="outsb", name="outsb")
                    for ib in range(NIB):
                        for g in range(NG):
                            av = pacc.tile([128, 128], F32, tag="acc", name="av")
                            for jt in range(NJT):
                                nc.tensor.matmul(
                                    av[:, 0:65],
                                    PTs[ib][jt][:, g * 128 : (g + 1) * 128],
                                    Vau[jt][:, h, :],
                                    start=(jt == 0),
                                    stop=(jt == NJT - 1),
                                )
                            rl = stg.tile([128, 1], F32, tag="rl", name="rl")
                            with nc.allow_low_precision("fp32 reciprocal"):
                                nc.vector.reciprocal(rl[:], av[:, 64:65])
                            nc.vector.tensor_scalar_mul(
                                outsb[:, ib * NG + g, :], av[:, 0:64], rl[:]
                            )
                    nc.sync.dma_start(
                        out=out_d.ap().rearrange("(a p) m -> p a m", p=128)[
                            :, :, h * D : (h + 1) * D
                        ],
                        in_=outsb[:],
                    )
    if not nc.is_finalized():
        nc.finalize()
    _CACHE["nc"] = nc
    return nc


def kernel(hidden_states, attention_mask, W_qkv):
    hs = np.asarray(hidden_states, dtype=np.float32)  # [2, 2048, 1024]
    am = np.asarray(attention_mask)  # [2, 2048]
    W = np.asarray(W_qkv, dtype=np.float32)  # [16, 1024, 192]

    nc = _build()
    in_maps = []
    for core in range(NCORES):
        b, hg = core // 4, core % 4
        Wc = W[hg * 4 : hg * 4 + 4]  # [4, 1024, 192]
        qk_cols = []
        for h in range(HPC):
            qk_cols.append(Wc[h, :, 0:64] * SCALE)  # q, pre-scaled
            qk_cols.append(Wc[h, :, 64:128])  # k
        aux = np.empty((2, NT), np.float32)
        aux[0] = ((am[b] != 0).astype(np.float32) - 1.0) * 30000.0
        aux[1] = 1.0
        in_maps.append(
            {
                "hsT": np.ascontiguousarray(hs[b].T).astype(bfloat16),
                "wqk": np.concatenate(qk_cols, axis=1).astype(bfloat16),
                "wv": np.concatenate(
                    [Wc[h, :, 128:192] for h in range(HPC)], axis=1
                ).astype(bfloat16),
                "aux": aux.astype(bfloat16),
            }
        )
    res = run_bass_kernel_spmd(nc, in_maps, list(range(NCORES)))
    if res.exec_time_ns is not None:
        print(f"HW exec time: {res.exec_time_ns} ns")
    if res.mean_exec_time_ns is not None:
        print(f"HW exec time (mean across cores): {res.mean_exec_time_ns} ns")
    out = np.empty((2, NT, HID), dtype=np.float32)
    for core in range(NCORES):
        b, hg = core // 4, core % 4
        out[b, :, hg * 256 : (hg + 1) * 256] = res.results[core]["out"]
    return out


def predicted_exec_ns():
    """Device-occupancy estimate for one core (all 8 run the same program
    in parallel). Used by test.py; the real NTFF profiling hook is not
    available in this container."""
    nc = _build()
    from concourse.timeline_sim import TimelineSim
    return float(TimelineSim(nc, trace=False).simulate())


# revision 3
# speedup vs baseline: 1.4362x; 1.0328x over previous
"""MHSA Trainium2 kernel: B=2, N=2048, H=1024, 16 heads x d=64, fp32 in/out.

Sharding: 8 cores = 2 (batch) x 4 (head-groups of 4 heads). Each core is
fully independent (no collectives); host gathers per-core [2048, 256]
outputs into [2, 2048, 1024].

Per-core structure (the scalar engine's exp stream is the critical path at
~133us; PE work is ~126us and is interleaved into the exp stream's slack):
  - All matmul operands bf16 (fp32 PSUM accumulation; fp32 normalize).
  - QK proj (W stationary): QT/KT in [65, head, tok] layout. Row 64 of KT
    holds the additive mask bias per key, row 64 of QT holds ones, so the
    scores matmul contracts K=65 and applies the mask for free (the 1/sqrt(H)
    scale is folded into W_q on the host).
  - V proj (tokens stationary): V_aug tiles [tok=128, head, 65] with a ones
    column; the AV matmul's output column 64 accumulates the softmax
    denominator.
  - scores^T[j, i] per (head, j-tile, i-halfblock) -> PSUM [128, 1024];
    exp via one scalar-engine ACT per tile -> bf16 P^T in SBUF.
  - AV in the [i, d] orientation: lhsT = P^T chunk [j=128, i=128], rhs =
    V_aug [j=128, 65], accumulated over 16 j-chunks -> out[i, 65].
  - normalize: DVE reciprocal of column 64 + tensor_scalar multiply; output
    DMA in natural [token, (h d)] layout.
  - Program order: scores/exp units are the backbone; projections of the
    next head, V-projection chunks, and AV groups of the previous i-block
    are sprinkled between units so every engine stays busy and the exp
    stream starts ~10us in (DMA-paced) and never starves.
"""

import numpy as np
from ml_dtypes import bfloat16

import concourse.bass as bass
import concourse.bacc as bacc
import concourse.mybir as mybir
import concourse.tile as tile
from concourse.bass_utils import run_bass_kernel_spmd

F32 = mybir.dt.float32
BF16 = mybir.dt.bfloat16
AF = mybir.ActivationFunctionType

HID = 1024
NT = 2048
D = 64
HPC = 4  # heads per core
NCORES = 8
SCALE = float(HID) ** -0.5
KD = HID // 128  # 8 contraction chunks
NJT = NT // 128  # 16 j-tiles
IB = 1024  # i-block (exp/PSUM unit)
NIB = NT // IB  # 2
NG = IB // 128  # 8 i-groups per i-block

_CACHE = {}


def _build():
    if "nc" in _CACHE:
        return _CACHE["nc"]
    nc = bacc.Bacc("TRN2", debug=False)
    hsT_d = nc.dram_tensor("hsT", [HID, NT], BF16, kind="ExternalInput")
    wqk_d = nc.dram_tensor("wqk", [HID, HPC * 128], BF16, kind="ExternalInput")
    wv_d = nc.dram_tensor("wv", [HID, HPC * D], BF16, kind="ExternalInput")
    aux_d = nc.dram_tensor("aux", [2, HPC, NT], BF16, kind="ExternalInput")
    out_d = nc.dram_tensor("out", [NT, HPC * D], F32, kind="ExternalOutput")

    with tile.TileContext(nc) as tc:
        with tc.tile_pool(name="per", bufs=1) as per:
            hsT = per.tile([128, KD, NT], BF16, tag="hst", name="hst")
            wqk = per.tile([128, KD, HPC * 128], BF16, tag="wqk", name="wqk")
            wv = per.tile([128, KD, HPC * D], BF16, tag="wv", name="wv")
            QT = per.tile([65, HPC, NT], BF16, tag="qt", name="qt")
            KT = per.tile([65, HPC, NT], BF16, tag="kt", name="kt")
            Vau = [per.tile([128, HPC, 65], BF16, tag=f"vau{t}", name=f"vau{t}") for t in range(NJT)]

            hsT_r = hsT_d.ap().rearrange("(c p) m -> p c m", p=128)
            wqk_r = wqk_d.ap().rearrange("(c p) m -> p c m", p=128)
            wv_r = wv_d.ap().rearrange("(c p) m -> p c m", p=128)
            # DMA order tracks first use: wqk, first token-half of hsT (QK
            # proj of head 0), mask/ones rows, rest of hsT, then wv.
            nc.sync.dma_start(out=wqk[:], in_=wqk_r)
            nc.sync.dma_start(out=hsT[:, :, 0:NT // 2], in_=hsT_r[:, :, 0:NT // 2])
            nc.sync.dma_start(out=KT[64:65, :, :], in_=aux_d.ap()[0:1])
            nc.sync.dma_start(out=QT[64:65, :, :], in_=aux_d.ap()[1:2])
            nc.sync.dma_start(out=hsT[:, :, NT // 2:NT], in_=hsT_r[:, :, NT // 2:NT])
            nc.sync.dma_start(out=wv[:], in_=wv_r)
            for t in range(NJT):
                nc.gpsimd.memset(Vau[t][:, :, 64:65], 1.0)

            with (
                tc.tile_pool(name="psqk", bufs=2, space="PSUM") as psqk,
                tc.tile_pool(name="pacc", bufs=2, space="PSUM") as pacc,
                tc.tile_pool(name="psc", bufs=2, space="PSUM") as psc,
                tc.tile_pool(name="ptp", bufs=2) as ptp,
                tc.tile_pool(name="stg", bufs=2) as stg,
            ):
                PTs = {}  # (h, ib) -> [pt tile per jt]
                outsb = {}  # (h, ib) -> staging tile

                def qk_chunk(h, t):
                    ts = slice(t * 512, (t + 1) * 512)
                    acc = psqk.tile([128, 512], F32, tag="qk", name="acc")
                    for k in range(KD):
                        nc.tensor.matmul(
                            acc[:],
                            wqk[:, k, h * 128 : (h + 1) * 128],
                            hsT[:, k, ts],
                            start=(k == 0),
                            stop=(k == KD - 1),
                        )
                    nc.vector.tensor_copy(QT[0:64, h, ts], acc[0:64, :])
                    nc.vector.tensor_copy(KT[0:64, h, ts], acc[64:128, :])

                def v_chunk(t):
                    pv = pacc.tile([128, HPC * D], F32, tag="acc", name="pv")
                    for k in range(KD):
                        nc.tensor.matmul(
                            pv[:],
                            hsT[:, k, t * 128 : (t + 1) * 128],
                            wv[:, k, :],
                            start=(k == 0),
                            stop=(k == KD - 1),
                        )
                    nc.vector.tensor_copy(
                        Vau[t][:, :, 0:64],
                        pv[:].rearrange("p (h d) -> p h d", h=HPC),
                    )

                def score_unit(h, ib, jt):
                    i0 = ib * IB
                    sc = psc.tile([128, IB], F32, tag="sc", name="sc")
                    for ic in range(IB // 512):
                        cs = slice(ic * 512, (ic + 1) * 512)
                        qs = slice(i0 + ic * 512, i0 + (ic + 1) * 512)
                        nc.tensor.matmul(
                            sc[:, cs],
                            KT[0:65, h, jt * 128 : (jt + 1) * 128],
                            QT[0:65, h, qs],
                            start=True,
                            stop=True,
                        )
                    pt = ptp.tile([128, IB], BF16, tag=f"pt{jt}", name="pt")
                    nc.scalar.activation(pt[:], sc[:], AF.Exp)
                    PTs[(h, ib)][jt] = pt

                def av_group(h, ib, g):
                    if g == 0:
                        outsb[(h, ib)] = stg.tile(
                            [128, NG, D], F32, tag="ob", name="ob", bufs=3
                        )
                    ob = outsb[(h, ib)]
                    av = pacc.tile([128, 128], F32, tag="acc", name="av")
                    for jt in range(NJT):
                        nc.tensor.matmul(
                            av[:, 0:65],
                            PTs[(h, ib)][jt][:, g * 128 : (g + 1) * 128],
                            Vau[jt][:, h, :],
                            start=(jt == 0),
                            stop=(jt == NJT - 1),
                        )
                    rl = stg.tile([128, 1], F32, tag="rl", name="rl")
                    with nc.allow_low_precision("fp32 reciprocal"):
                        nc.vector.reciprocal(rl[:], av[:, 64:65])
                    nc.vector.tensor_scalar_mul(ob[:, g, :], av[:, 0:64], rl[:])
                    if g == NG - 1:
                        del PTs[(h, ib)]
                        nc.sync.dma_start(
                            out=out_d.ap().rearrange("(a p) m -> p a m", p=128)[
                                :, ib * NG : (ib + 1) * NG, h * D : (h + 1) * D
                            ],
                            in_=ob[:],
                        )
                        del outsb[(h, ib)]

                # Per-head extras schedule: unit u (= ib*16+jt) -> closures
                # sprinkled after that scores unit. Placement tracks data
                # arrival (DMA halves) and ACT progress (AV after its
                # i-block's exps are done; next head's proj under this
                # head's exp stream).
                def extras_for(h):
                    ex = {}
                    if h == 0:
                        ex[5] = [lambda: qk_chunk(0, 2)]
                        ex[7] = [lambda: qk_chunk(0, 3)]
                        ex[9] = [lambda: qk_chunk(1, 0)]
                        ex[11] = [lambda: qk_chunk(1, 1)]
                        ex[13] = [lambda: qk_chunk(1, 2), lambda: v_chunk(0)]
                        ex[14] = [lambda: v_chunk(1), lambda: v_chunk(2)]
                        ex[15] = [lambda: v_chunk(3), lambda: v_chunk(4)]
                        ex[16] = [lambda: qk_chunk(1, 3), lambda: v_chunk(5)]
                        for i in range(5):
                            ex[17 + i] = [
                                (lambda t: lambda: v_chunk(t))(6 + 2 * i),
                                (lambda t: lambda: v_chunk(t))(7 + 2 * i),
                            ]
                        for g in range(NG):
                            ex[24 + g] = [(lambda gg: lambda: av_group(0, 0, gg))(g)]
                    else:
                        for g in range(NG):
                            ex[2 + g] = [
                                (lambda gg: lambda: av_group(h - 1, 1, gg))(g)
                            ]
                        if h + 1 < HPC:
                            for t in range(4):
                                ex[10 + 2 * t] = [
                                    (lambda tt: lambda: qk_chunk(h + 1, tt))(t)
                                ]
                        for g in range(NG):
                            ex[18 + g] = [(lambda gg: lambda: av_group(h, 0, gg))(g)]
                    return ex

                qk_chunk(0, 0)
                qk_chunk(0, 1)
                for h in range(HPC):
                    ex = extras_for(h)
                    for ib in range(NIB):
                        PTs[(h, ib)] = [None] * NJT
                        for jt in range(NJT):
                            score_unit(h, ib, jt)
                            for fn in ex.get(ib * NJT + jt, ()):
                                fn()
                for g in range(NG):
                    av_group(HPC - 1, 1, g)
    if not nc.is_finalized():
        nc.finalize()
    _CACHE["nc"] = nc
    return nc


def kernel(hidden_states, attention_mask, W_qkv):
    hs = np.asarray(hidden_states, dtype=np.float32)  # [2, 2048, 1024]
    am = np.asarray(attention_mask)  # [2, 2048]
    W = np.asarray(W_qkv, dtype=np.float32)  # [16, 1024, 192]

    nc = _build()
    in_maps = []
    for core in range(NCORES):
        b, hg = core // 4, core % 4
        Wc = W[hg * 4 : hg * 4 + 4]  # [4, 1024, 192]
        qk_cols = []
        for h in range(HPC):
            qk_cols.append(Wc[h, :, 0:64] * SCALE)  # q, pre-scaled
            qk_cols.append(Wc[h, :, 64:128])  # k
        aux = np.empty((2, HPC, NT), np.float32)
        aux[0] = (((am[b] != 0).astype(np.float32) - 1.0) * 30000.0)[None, :]
        aux[1] = 1.0
        in_maps.append(
            {
                "hsT": np.ascontiguousarray(hs[b].T).astype(bfloat16),
                "wqk": np.concatenate(qk_cols, axis=1).astype(bfloat16),
                "wv": np.concatenate(
                    [Wc[h, :, 128:192] for h in range(HPC)], axis=1
                ).astype(bfloat16),
                "aux": aux.astype(bfloat16),
            }
        )
    res = run_bass_kernel_spmd(nc, in_maps, list(range(NCORES)))
    if res.exec_time_ns is not None:
        print(f"HW exec time: {res.exec_time_ns} ns")
    if res.mean_exec_time_ns is not None:
        print(f"HW exec time (mean across cores): {res.mean_exec_time_ns} ns")
    out = np.empty((2, NT, HID), dtype=np.float32)
    for core in range(NCORES):
        b, hg = core // 4, core % 4
        out[b, :, hg * 256 : (hg + 1) * 256] = res.results[core]["out"]
    return out


def predicted_exec_ns():
    """Device-occupancy estimate for one core (all 8 run the same program
    in parallel). Used by test.py; the real NTFF profiling hook is not
    available in this container."""
    nc = _build()
    from concourse.timeline_sim import TimelineSim
    return float(TimelineSim(nc, trace=False).simulate())


# revision 7
# speedup vs baseline: 1.6017x; 1.1153x over previous
"""MHSA Trainium2 kernel: B=2, N=2048, H=1024, 16 heads x d=64, fp32 in/out.

Sharding: 8 cores = 2 (batch) x 4 (head-groups of 4 heads). Each core is
fully independent (no collectives); host gathers per-core [2048, 256]
outputs into [2, 2048, 1024].

Per-core structure (the scalar engine's exp stream is the critical path at
~133us; PE work is ~126us and is interleaved into the exp stream's slack):
  - All matmul operands bf16 (fp32 PSUM accumulation; fp32 normalize).
  - QK proj (W stationary): QT/KT in [65, head, tok] layout. Row 64 of KT
    holds the additive mask bias per key, row 64 of QT holds ones, so the
    scores matmul contracts K=65 and applies the mask for free (the 1/sqrt(H)
    scale is folded into W_q on the host).
  - V proj (tokens stationary): V_aug tiles [tok=128, head, 65] with a ones
    column; the AV matmul's output column 64 accumulates the softmax
    denominator.
  - scores^T[j, i] per (head, j-tile, i-halfblock) -> PSUM [128, 1024];
    exp via one scalar-engine ACT per tile -> bf16 P^T in SBUF.
  - AV in the [i, d] orientation: lhsT = P^T chunk [j=128, i=128], rhs =
    V_aug [j=128, 65], accumulated over 16 j-chunks -> out[i, 65].
  - normalize: DVE reciprocal of column 64 + tensor_scalar multiply; output
    DMA in natural [token, (h d)] layout.
  - Program order: scores/exp units are the backbone; projections of the
    next head, V-projection chunks, and AV groups of the previous i-block
    are sprinkled between units so every engine stays busy and the exp
    stream starts ~10us in (DMA-paced) and never starves.
"""

import numpy as np
from ml_dtypes import bfloat16

import concourse.bass as bass
import concourse.bacc as bacc
import concourse.mybir as mybir
import concourse.tile as tile
from concourse.bass_utils import run_bass_kernel_spmd

F32 = mybir.dt.float32
BF16 = mybir.dt.bfloat16
AF = mybir.ActivationFunctionType

HID = 1024
NT = 2048
D = 64
HPC = 4  # heads per core
NCORES = 8
SCALE = float(HID) ** -0.5
KD = HID // 128  # 8 contraction chunks
NJT = NT // 128  # 16 j-tiles
IB = 1024  # i-block (exp/PSUM unit)
NIB = NT // IB  # 2
NG = IB // 128  # 8 i-groups per i-block

_CACHE = {}


def _build():
    if "nc" in _CACHE:
        return _CACHE["nc"]
    nc = bacc.Bacc("TRN2", debug=False)
    hsT_d = nc.dram_tensor("hsT", [HID, NT], BF16, kind="ExternalInput")
    wqk_d = nc.dram_tensor("wqk", [HID, HPC * 128], BF16, kind="ExternalInput")
    wv_d = nc.dram_tensor("wv", [HID, HPC * D], BF16, kind="ExternalInput")
    aux_d = nc.dram_tensor("aux", [2, HPC, NT], BF16, kind="ExternalInput")
    out_d = nc.dram_tensor("out", [NT, HPC * D], F32, kind="ExternalOutput")

    with tile.TileContext(nc) as tc:
        with tc.tile_pool(name="per", bufs=1) as per:
            hsT = per.tile([128, KD, NT], BF16, tag="hst", name="hst")
            wqk = per.tile([128, KD, HPC * 128], BF16, tag="wqk", name="wqk")
            wv = per.tile([128, KD, HPC * D], BF16, tag="wv", name="wv")
            QT = per.tile([65, HPC, NT], BF16, tag="qt", name="qt")
            KT = per.tile([65, HPC, NT], BF16, tag="kt", name="kt")
            Vau = [per.tile([128, HPC, 65], BF16, tag=f"vau{t}", name=f"vau{t}") for t in range(NJT)]

            scratch = per.tile([128, 640], BF16, tag="scr", name="scratch")
            nc.vector.memset(scratch[:], 0.0)

            hsT_r = hsT_d.ap().rearrange("(c p) m -> p c m", p=128)
            wqk_r = wqk_d.ap().rearrange("(c p) m -> p c m", p=128)
            wv_r = wv_d.ap().rearrange("(c p) m -> p c m", p=128)
            # DMA order tracks first use: head-0 W columns, first two token
            # quarters of hsT (QK proj of head 0), mask/ones rows, the rest.
            nc.sync.dma_start(out=wqk[:, :, 0:128], in_=wqk_r[:, :, 0:128])
            nc.sync.dma_start(out=hsT[:, :, 0:512], in_=hsT_r[:, :, 0:512])
            nc.sync.dma_start(out=hsT[:, :, 512:1024], in_=hsT_r[:, :, 512:1024])
            nc.sync.dma_start(out=KT[64:65, :, :], in_=aux_d.ap()[0:1])
            nc.sync.dma_start(out=QT[64:65, :, :], in_=aux_d.ap()[1:2])
            nc.sync.dma_start(out=wqk[:, :, 128:512], in_=wqk_r[:, :, 128:512])
            nc.sync.dma_start(out=hsT[:, :, NT // 2:NT], in_=hsT_r[:, :, NT // 2:NT])
            nc.sync.dma_start(out=wv[:], in_=wv_r)
            for t in range(NJT):
                nc.gpsimd.memset(Vau[t][:, :, 64:65], 1.0)

            with (
                tc.tile_pool(name="psqk", bufs=2, space="PSUM") as psqk,
                tc.tile_pool(name="pacc", bufs=2, space="PSUM") as pacc,
                tc.tile_pool(name="psc", bufs=2, space="PSUM") as psc,
                tc.tile_pool(name="ptp", bufs=3) as ptp,
                tc.tile_pool(name="stg", bufs=2) as stg,
            ):
                # PE p-state warmup: the clock ramps to full rate only after
                # ~3us of continuous matmul activity, and the input DMAs take
                # ~10us anyway — burn the ramp on scratch matmuls.
                for w in range(24):
                    wacc = psqk.tile([128, 512], F32, tag="qk", name="wacc")
                    nc.tensor.matmul(
                        wacc[:], scratch[:, 0:128], scratch[:, 128:640],
                        start=True, stop=True,
                    )
                PTs = {}  # (h, ib) -> [pt tile per jt]
                outsb = {}  # (h, ib) -> staging tile

                def qk_chunk(h, t):
                    ts = slice(t * 512, (t + 1) * 512)
                    acc = psqk.tile([128, 512], F32, tag="qk", name="acc")
                    for k in range(KD):
                        nc.tensor.matmul(
                            acc[:],
                            wqk[:, k, h * 128 : (h + 1) * 128],
                            hsT[:, k, ts],
                            start=(k == 0),
                            stop=(k == KD - 1),
                        )
                    nc.vector.tensor_copy(QT[0:64, h, ts], acc[0:64, :])
                    nc.vector.tensor_copy(KT[0:64, h, ts], acc[64:128, :])

                def v_chunk(t):
                    pv = pacc.tile([128, HPC * D], F32, tag="acc", name="pv")
                    for k in range(KD):
                        nc.tensor.matmul(
                            pv[:],
                            hsT[:, k, t * 128 : (t + 1) * 128],
                            wv[:, k, :],
                            start=(k == 0),
                            stop=(k == KD - 1),
                        )
                    nc.vector.tensor_copy(
                        Vau[t][:, :, 0:64],
                        pv[:].rearrange("p (h d) -> p h d", h=HPC),
                    )

                def score_unit(h, ib, jt):
                    i0 = ib * IB
                    sc = psc.tile([128, IB], F32, tag="sc", name="sc")
                    for ic in range(IB // 512):
                        cs = slice(ic * 512, (ic + 1) * 512)
                        qs = slice(i0 + ic * 512, i0 + (ic + 1) * 512)
                        nc.tensor.matmul(
                            sc[:, cs],
                            KT[0:65, h, jt * 128 : (jt + 1) * 128],
                            QT[0:65, h, qs],
                            start=True,
                            stop=True,
                        )
                    pt = ptp.tile([128, IB], BF16, tag=f"pt{jt}", name="pt")
                    nc.scalar.activation(pt[:], sc[:], AF.Exp)
                    PTs[(h, ib)][jt] = pt

                def av_group(h, ib, g):
                    if g == 0:
                        outsb[(h, ib)] = stg.tile(
                            [128, NG, D], F32, tag="ob", name="ob", bufs=3
                        )
                    ob = outsb[(h, ib)]
                    av = pacc.tile([128, 128], F32, tag="acc", name="av")
                    jts = [(2 * g + i) % NJT for i in range(NJT)]
                    for i, jt in enumerate(jts):
                        nc.tensor.matmul(
                            av[:, 0:65],
                            PTs[(h, ib)][jt][:, g * 128 : (g + 1) * 128],
                            Vau[jt][:, h, :],
                            start=(i == 0),
                            stop=(i == NJT - 1),
                        )
                    rl = stg.tile([128, 1], F32, tag="rl", name="rl")
                    with nc.allow_low_precision("fp32 reciprocal"):
                        nc.vector.reciprocal(rl[:], av[:, 64:65])
                    nc.vector.tensor_scalar_mul(ob[:, g, :], av[:, 0:64], rl[:])
                    if g == NG // 2 - 1 or g == NG - 1:
                        gs = slice(0, NG // 2) if g < NG // 2 else slice(NG // 2, NG)
                        nc.sync.dma_start(
                            out=out_d.ap().rearrange("(a p) m -> p a m", p=128)[
                                :,
                                ib * NG + gs.start : ib * NG + gs.stop,
                                h * D : (h + 1) * D,
                            ],
                            in_=ob[:, gs, :],
                        )
                    if g == NG - 1:
                        del PTs[(h, ib)]
                        del outsb[(h, ib)]

                # Per-head extras schedule: unit u (= ib*16+jt) -> closures
                # sprinkled after that scores unit. Placement tracks data
                # arrival (DMA halves) and ACT progress (AV after its
                # i-block's exps are done; next head's proj under this
                # head's exp stream).
                def extras_for(h):
                    # AV work of head h-1 and the projection of head h+1 ride
                    # under head h's exp stream; V-projection chunks ride
                    # under head 0's.
                    ex = {}
                    qk = lambda hh, tt: (lambda: qk_chunk(hh, tt))
                    av = lambda hh, ib, gg: (lambda: av_group(hh, ib, gg))
                    vp = lambda tt: (lambda: v_chunk(tt))
                    if h == 0:
                        ex[5] = [qk(0, 2)]
                        ex[7] = [qk(0, 3)]
                        ex[9] = [qk(1, 0)]
                        ex[11] = [qk(1, 1)]
                        for t in range(NJT):
                            ex[12 + t] = [vp(t)]
                    elif h < HPC - 1:
                        hp = h + 1
                        ex[2] = [qk(h, 2)] if h == 1 else [av(h - 1, 0, 0)]
                        ex[4] = [qk(h, 3)] if h == 1 else [av(h - 1, 0, 1)]
                        if h == 1:
                            for g in range(NG):
                                ex[5 + g] = [av(0, 0, g)]
                        else:
                            for g in range(2, NG):
                                ex[3 + g] = [av(h - 1, 0, g)]
                        for t in range(4):
                            ex[13 + 2 * t] = [qk(hp, t)]
                        for g in range(NG):
                            ex[20 + g] = [av(h - 1, 1, g)]
                    else:
                        for g in range(NG):
                            ex[2 + g] = [av(h - 1, 0, g)]
                        for g in range(NG):
                            ex[11 + g] = [av(h - 1, 1, g)]
                        for g in range(NG):
                            ex[20 + g] = [av(h, 0, g)]
                    return ex

                qk_chunk(0, 0)
                qk_chunk(0, 1)
                for h in range(HPC):
                    ex = extras_for(h)
                    for ib in range(NIB):
                        PTs[(h, ib)] = [None] * NJT
                        for jt in range(NJT):
                            score_unit(h, ib, jt)
                            for fn in ex.get(ib * NJT + jt, ()):
                                fn()
                for g in range(NG):
                    av_group(HPC - 1, 1, g)
    if not nc.is_finalized():
        nc.finalize()
    _CACHE["nc"] = nc
    return nc


def kernel(hidden_states, attention_mask, W_qkv):
    hs = np.asarray(hidden_states, dtype=np.float32)  # [2, 2048, 1024]
    am = np.asarray(attention_mask)  # [2, 2048]
    W = np.asarray(W_qkv, dtype=np.float32)  # [16, 1024, 192]

    nc = _build()
    in_maps = []
    for core in range(NCORES):
        b, hg = core // 4, core % 4
        Wc = W[hg * 4 : hg * 4 + 4]  # [4, 1024, 192]
        qk_cols = []
        for h in range(HPC):
            qk_cols.append(Wc[h, :, 0:64] * SCALE)  # q, pre-scaled
            qk_cols.append(Wc[h, :, 64:128])  # k
        aux = np.empty((2, HPC, NT), np.float32)
        aux[0] = (((am[b] != 0).astype(np.float32) - 1.0) * 30000.0)[None, :]
        aux[1] = 1.0
        in_maps.append(
            {
                "hsT": np.ascontiguousarray(hs[b].T).astype(bfloat16),
                "wqk": np.concatenate(qk_cols, axis=1).astype(bfloat16),
                "wv": np.concatenate(
                    [Wc[h, :, 128:192] for h in range(HPC)], axis=1
                ).astype(bfloat16),
                "aux": aux.astype(bfloat16),
            }
        )
    res = run_bass_kernel_spmd(nc, in_maps, list(range(NCORES)))
    if res.exec_time_ns is not None:
        print(f"HW exec time: {res.exec_time_ns} ns")
    if res.mean_exec_time_ns is not None:
        print(f"HW exec time (mean across cores): {res.mean_exec_time_ns} ns")
    out = np.empty((2, NT, HID), dtype=np.float32)
    for core in range(NCORES):
        b, hg = core // 4, core % 4
        out[b, :, hg * 256 : (hg + 1) * 256] = res.results[core]["out"]
    return out


def predicted_exec_ns():
    """Device-occupancy estimate for one core (all 8 run the same program
    in parallel). Used by test.py; the real NTFF profiling hook is not
    available in this container."""
    nc = _build()
    from concourse.timeline_sim import TimelineSim
    return float(TimelineSim(nc, trace=False).simulate())


# revision 12
# speedup vs baseline: 1.6103x; 1.0054x over previous
"""MHSA Trainium2 kernel: B=2, N=2048, H=1024, 16 heads x d=64, fp32 in/out.

Sharding: 8 cores = 2 (batch) x 4 (head-groups of 4 heads). Each core is
fully independent (no collectives); host gathers per-core [2048, 256]
outputs into [2, 2048, 1024].

Per-core structure (the scalar engine's exp stream is the critical path at
~133us; PE work is ~126us and is interleaved into the exp stream's slack):
  - All matmul operands bf16 (fp32 PSUM accumulation; fp32 normalize).
  - QK proj (W stationary): QT/KT in [65, head, tok] layout. Row 64 of KT
    holds the additive mask bias per key, row 64 of QT holds ones, so the
    scores matmul contracts K=65 and applies the mask for free (the 1/sqrt(H)
    scale is folded into W_q on the host).
  - V proj (tokens stationary): V_aug tiles [tok=128, head, 65] with a ones
    column; the AV matmul's output column 64 accumulates the softmax
    denominator.
  - scores^T[j, i] per (head, j-tile, i-halfblock) -> PSUM [128, 1024];
    exp via one scalar-engine ACT per tile -> bf16 P^T in SBUF.
  - AV in the [i, d] orientation: lhsT = P^T chunk [j=128, i=128], rhs =
    V_aug [j=128, 65], accumulated over 16 j-chunks -> out[i, 65].
  - normalize: DVE reciprocal of column 64 + tensor_scalar multiply; output
    DMA in natural [token, (h d)] layout.
  - Program order: scores/exp units are the backbone; projections of the
    next head, V-projection chunks, and AV groups of the previous i-block
    are sprinkled between units so every engine stays busy and the exp
    stream starts ~10us in (DMA-paced) and never starves.
"""

import numpy as np
from ml_dtypes import bfloat16

import concourse.bass as bass
import concourse.bacc as bacc
import concourse.mybir as mybir
import concourse.tile as tile
from concourse.bass_utils import run_bass_kernel_spmd

F32 = mybir.dt.float32
BF16 = mybir.dt.bfloat16
AF = mybir.ActivationFunctionType

HID = 1024
NT = 2048
D = 64
HPC = 4  # heads per core
NCORES = 8
SCALE = float(HID) ** -0.5
KD = HID // 128  # 8 contraction chunks
NJT = NT // 128  # 16 j-tiles
IB = 1024  # i-block (exp/PSUM unit)
NIB = NT // IB  # 2
NG = IB // 128  # 8 i-groups per i-block

_CACHE = {}


def _build():
    if "nc" in _CACHE:
        return _CACHE["nc"]
    nc = bacc.Bacc("TRN2", debug=False)
    hsT_d = nc.dram_tensor("hsT", [HID, NT], BF16, kind="ExternalInput")
    wqk_d = nc.dram_tensor("wqk", [HID, HPC * 128], BF16, kind="ExternalInput")
    wv_d = nc.dram_tensor("wv", [HID, HPC * D], BF16, kind="ExternalInput")
    aux_d = nc.dram_tensor("aux", [2, HPC, NT], BF16, kind="ExternalInput")
    out_d = nc.dram_tensor("out", [NT, HPC * D], F32, kind="ExternalOutput")

    with tile.TileContext(nc) as tc:
        with tc.tile_pool(name="per", bufs=1) as per:
            hsT = per.tile([128, KD, NT], BF16, tag="hst", name="hst")
            wqk = per.tile([128, KD, HPC * 128], BF16, tag="wqk", name="wqk")
            wv = per.tile([128, KD, HPC * D], BF16, tag="wv", name="wv")
            QT = per.tile([65, HPC, NT], BF16, tag="qt", name="qt")
            KT = per.tile([65, HPC, NT], BF16, tag="kt", name="kt")
            Vau = [per.tile([128, HPC, 65], BF16, tag=f"vau{t}", name=f"vau{t}") for t in range(NJT)]

            scratch = per.tile([128, 640], BF16, tag="scr", name="scratch")
            nc.vector.memset(scratch[:], 0.0)

            hsT_r = hsT_d.ap().rearrange("(c p) m -> p c m", p=128)
            wqk_r = wqk_d.ap().rearrange("(c p) m -> p c m", p=128)
            wv_r = wv_d.ap().rearrange("(c p) m -> p c m", p=128)
            # DMA order tracks first use: head-0 W columns, first two token
            # quarters of hsT (QK proj of head 0), mask/ones rows, the rest.
            nc.sync.dma_start(out=wqk[:, :, 0:128], in_=wqk_r[:, :, 0:128])
            nc.sync.dma_start(out=hsT[:, :, 0:512], in_=hsT_r[:, :, 0:512])
            nc.sync.dma_start(out=hsT[:, :, 512:1024], in_=hsT_r[:, :, 512:1024])
            nc.sync.dma_start(out=KT[64:65, :, :], in_=aux_d.ap()[0:1])
            nc.sync.dma_start(out=QT[64:65, :, :], in_=aux_d.ap()[1:2])
            nc.sync.dma_start(out=wqk[:, :, 128:512], in_=wqk_r[:, :, 128:512])
            nc.sync.dma_start(out=hsT[:, :, NT // 2:NT], in_=hsT_r[:, :, NT // 2:NT])
            nc.sync.dma_start(out=wv[:], in_=wv_r)
            for t in range(NJT):
                nc.gpsimd.memset(Vau[t][:, :, 64:65], 1.0)

            with (
                tc.tile_pool(name="psqk", bufs=2, space="PSUM") as psqk,
                tc.tile_pool(name="pacc", bufs=2, space="PSUM") as pacc,
                tc.tile_pool(name="psc", bufs=2, space="PSUM") as psc,
                tc.tile_pool(name="ptp", bufs=3) as ptp,
                tc.tile_pool(name="stg", bufs=2) as stg,
            ):
                # PE p-state warmup: the clock ramps to full rate only after
                # ~3us of continuous matmul activity, and the input DMAs take
                # ~7us anyway — burn the ramp on scratch matmuls.
                for w in range(18):
                    wacc = psqk.tile([128, 512], F32, tag="qk", name="wacc")
                    nc.tensor.matmul(
                        wacc[:], scratch[:, 0:128], scratch[:, 128:640],
                        start=True, stop=True,
                    )
                PTs = {}  # (h, ib) -> [pt tile per jt]
                outsb = {}  # (h, ib) -> staging tile

                def qk_chunk(h, t):
                    ts = slice(t * 512, (t + 1) * 512)
                    acc = psqk.tile([128, 512], F32, tag="qk", name="acc")
                    for k in range(KD):
                        nc.tensor.matmul(
                            acc[:],
                            wqk[:, k, h * 128 : (h + 1) * 128],
                            hsT[:, k, ts],
                            start=(k == 0),
                            stop=(k == KD - 1),
                        )
                    nc.vector.tensor_copy(QT[0:64, h, ts], acc[0:64, :])
                    nc.vector.tensor_copy(KT[0:64, h, ts], acc[64:128, :])

                def v_chunk(t):
                    pv = pacc.tile([128, HPC * D], F32, tag="acc", name="pv")
                    for k in range(KD):
                        nc.tensor.matmul(
                            pv[:],
                            hsT[:, k, t * 128 : (t + 1) * 128],
                            wv[:, k, :],
                            start=(k == 0),
                            stop=(k == KD - 1),
                        )
                    nc.vector.tensor_copy(
                        Vau[t][:, :, 0:64],
                        pv[:].rearrange("p (h d) -> p h d", h=HPC),
                    )

                def score_unit(h, ib, jt):
                    i0 = ib * IB
                    sc = psc.tile([128, IB], F32, tag="sc", name="sc")
                    for ic in range(IB // 512):
                        cs = slice(ic * 512, (ic + 1) * 512)
                        qs = slice(i0 + ic * 512, i0 + (ic + 1) * 512)
                        nc.tensor.matmul(
                            sc[:, cs],
                            KT[0:65, h, jt * 128 : (jt + 1) * 128],
                            QT[0:65, h, qs],
                            start=True,
                            stop=True,
                        )
                    pt = ptp.tile([128, IB], BF16, tag=f"pt{jt}", name="pt")
                    nc.scalar.activation(pt[:], sc[:], AF.Exp)
                    PTs[(h, ib)][jt] = pt

                def av_group(h, ib, g):
                    if g == 0:
                        outsb[(h, ib)] = stg.tile(
                            [128, NG, D], F32, tag="ob", name="ob", bufs=3
                        )
                    ob = outsb[(h, ib)]
                    av = pacc.tile([128, 128], F32, tag="acc", name="av")
                    jts = [(2 * g + i) % NJT for i in range(NJT)]
                    for i, jt in enumerate(jts):
                        nc.tensor.matmul(
                            av[:, 0:65],
                            PTs[(h, ib)][jt][:, g * 128 : (g + 1) * 128],
                            Vau[jt][:, h, :],
                            start=(i == 0),
                            stop=(i == NJT - 1),
                        )
                    rl = stg.tile([128, 1], F32, tag="rl", name="rl")
                    with nc.allow_low_precision("fp32 reciprocal"):
                        nc.vector.reciprocal(rl[:], av[:, 64:65])
                    nc.vector.tensor_scalar_mul(ob[:, g, :], av[:, 0:64], rl[:])
                    if g == NG // 2 - 1 or g == NG - 1:
                        gs = slice(0, NG // 2) if g < NG // 2 else slice(NG // 2, NG)
                        nc.sync.dma_start(
                            out=out_d.ap().rearrange("(a p) m -> p a m", p=128)[
                                :,
                                ib * NG + gs.start : ib * NG + gs.stop,
                                h * D : (h + 1) * D,
                            ],
                            in_=ob[:, gs, :],
                        )
                    if g == NG - 1:
                        del PTs[(h, ib)]
                        del outsb[(h, ib)]

                # Per-head extras schedule: unit u (= ib*16+jt) -> closures
                # sprinkled after that scores unit. Placement tracks data
                # arrival (DMA halves) and ACT progress (AV after its
                # i-block's exps are done; next head's proj under this
                # head's exp stream).
                def extras_for(h):
                    # AV work of head h-1 and the projection of head h+1 ride
                    # under head h's exp stream; V-projection chunks ride
                    # under head 0's. The last head's final i-block AV is
                    # interleaved batch-wise instead (see below).
                    ex = {}
                    qk = lambda hh, tt: (lambda: qk_chunk(hh, tt))
                    av = lambda hh, ib, gg: (lambda: av_group(hh, ib, gg))
                    vp = lambda tt: (lambda: v_chunk(tt))
                    if h == 0:
                        ex[5] = [qk(0, 2)]
                        ex[7] = [qk(0, 3)]
                        ex[9] = [qk(1, 0)]
                        ex[11] = [qk(1, 1)]
                        for t in range(14):
                            ex[12 + t] = [vp(t)]
                    elif h < HPC - 1:
                        ex[0] = [qk(h, 2)]
                        ex[2] = [qk(h, 3)]
                        if h == 1:
                            ex[1] = [vp(14)]
                            ex[3] = [vp(15)]
                        for g in range(NG):
                            ex[4 + g] = [av(h - 1, 0, g)]
                        ex[13] = [qk(h + 1, 0)]
                        ex[15] = [qk(h + 1, 1)]
                        for g in range(NG):
                            ex[17 + g] = [av(h - 1, 1, g)]
                    else:
                        ex[0] = [qk(h, 2)]
                        ex[2] = [qk(h, 3)]
                        for g in range(NG):
                            ex[3 + g] = [av(h - 1, 0, g)]
                        for g in range(NG):
                            ex[11 + g] = [av(h - 1, 1, g)]
                        for g in range(NG):
                            ex[20 + g] = [av(h, 0, g)]
                    return ex

                qk_chunk(0, 0)
                qk_chunk(0, 1)
                hl = HPC - 1
                av8 = [None, None]
                ob_last = None
                for h in range(HPC):
                    ex = extras_for(h)
                    for ib in range(NIB):
                        PTs[(h, ib)] = [None] * NJT
                        for jt in range(NJT):
                            score_unit(h, ib, jt)
                            for fn in ex.get(ib * NJT + jt, ()):
                                fn()
                            if h == hl and ib == 1:
                                # final i-block: AV batches ride 2 units
                                # behind the exp stream (all 8 i-groups
                                # accumulate in parallel in 2 psum banks)
                                if jt == 0:
                                    av8[0] = psqk.tile(
                                        [128, 512], F32, tag="qk", name="av8a"
                                    )
                                    av8[1] = psqk.tile(
                                        [128, 512], F32, tag="qk", name="av8b"
                                    )
                                    ob_last = stg.tile(
                                        [128, NG, D], F32, tag="ob", name="ob", bufs=3
                                    )
                                bjts = [jt - 2] if jt >= 2 else []
                                if jt == NJT - 1:
                                    bjts = [NJT - 3, NJT - 2, NJT - 1]
                                for bjt in bjts:
                                    for g in range(NG):
                                        q = (g % 4) * 128
                                        # start=True clears has_written for
                                        # the WHOLE bank, so only the first
                                        # group per bank may issue it; the
                                        # other groups' first matmuls
                                        # overwrite (bit unset) and then
                                        # accumulate.
                                        nc.tensor.matmul(
                                            av8[g // 4][:, q : q + 65],
                                            PTs[(hl, 1)][bjt][
                                                :, g * 128 : (g + 1) * 128
                                            ],
                                            Vau[bjt][:, hl, :],
                                            start=(bjt == 0 and g % 4 == 0),
                                            stop=(bjt == NJT - 1),
                                            skip_group_check=True,
                                        )
                # tail: normalize the final i-block (reciprocal on DVE; the
                # multiplies alternate DVE / scalar engine, both idle here)
                for g in range(NG):
                    q = (g % 4) * 128
                    rl = stg.tile([128, 1], F32, tag="rl", name="rl")
                    with nc.allow_low_precision("fp32 reciprocal"):
                        nc.vector.reciprocal(rl[:], av8[g // 4][:, q + 64 : q + 65])
                    if g % 2 == 0:
                        nc.scalar.mul(ob_last[:, g, :], av8[g // 4][:, q : q + 64], rl[:])
                    else:
                        nc.vector.tensor_scalar_mul(
                            ob_last[:, g, :], av8[g // 4][:, q : q + 64], rl[:]
                        )
                    if g == NG // 2 - 1 or g == NG - 1:
                        gs = slice(0, NG // 2) if g < NG // 2 else slice(NG // 2, NG)
                        nc.sync.dma_start(
                            out=out_d.ap().rearrange("(a p) m -> p a m", p=128)[
                                :,
                                NG + gs.start : NG + gs.stop,
                                hl * D : (hl + 1) * D,
                            ],
                            in_=ob_last[:, gs, :],
                        )
    if not nc.is_finalized():
        nc.finalize()
    _CACHE["nc"] = nc
    return nc


def kernel(hidden_states, attention_mask, W_qkv):
    hs = np.asarray(hidden_states, dtype=np.float32)  # [2, 2048, 1024]
    am = np.asarray(attention_mask)  # [2, 2048]
    W = np.asarray(W_qkv, dtype=np.float32)  # [16, 1024, 192]

    nc = _build()
    in_maps = []
    for core in range(NCORES):
        b, hg = core // 4, core % 4
        Wc = W[hg * 4 : hg * 4 + 4]  # [4, 1024, 192]
        qk_cols = []
        for h in range(HPC):
            qk_cols.append(Wc[h, :, 0:64] * SCALE)  # q, pre-scaled
            qk_cols.append(Wc[h, :, 64:128])  # k
        aux = np.empty((2, HPC, NT), np.float32)
        aux[0] = (((am[b] != 0).astype(np.float32) - 1.0) * 30000.0)[None, :]
        aux[1] = 1.0
        in_maps.append(
            {
                "hsT": np.ascontiguousarray(hs[b].T).astype(bfloat16),
                "wqk": np.concatenate(qk_cols, axis=1).astype(bfloat16),
                "wv": np.concatenate(
                    [Wc[h, :, 128:192] for h in range(HPC)], axis=1
                ).astype(bfloat16),
                "aux": aux.astype(bfloat16),
            }
        )
    res = run_bass_kernel_spmd(nc, in_maps, list(range(NCORES)))
    if res.exec_time_ns is not None:
        print(f"HW exec time: {res.exec_time_ns} ns")
    if res.mean_exec_time_ns is not None:
        print(f"HW exec time (mean across cores): {res.mean_exec_time_ns} ns")
    out = np.empty((2, NT, HID), dtype=np.float32)
    for core in range(NCORES):
        b, hg = core // 4, core % 4
        out[b, :, hg * 256 : (hg + 1) * 256] = res.results[core]["out"]
    return out


def predicted_exec_ns():
    """Device-occupancy estimate for one core (all 8 run the same program
    in parallel). Used by test.py; the real NTFF profiling hook is not
    available in this container."""
    nc = _build()
    from concourse.timeline_sim import TimelineSim
    return float(TimelineSim(nc, trace=False).simulate())


# revision 13
# speedup vs baseline: 1.6303x; 1.0124x over previous
"""MHSA Trainium2 kernel: B=2, N=2048, H=1024, 16 heads x d=64, fp32 in/out.

Sharding: 8 cores = 2 (batch) x 4 (head-groups of 4 heads). Each core is
fully independent (no collectives); host gathers per-core [2048, 256]
outputs into [2, 2048, 1024].

Per-core structure (the scalar engine's exp stream is the critical path at
~133us; PE work is ~126us and is interleaved into the exp stream's slack):
  - All matmul operands bf16 (fp32 PSUM accumulation; fp32 normalize).
  - QK proj (W stationary): QT/KT in [65, head, tok] layout. Row 64 of KT
    holds the additive mask bias per key, row 64 of QT holds ones, so the
    scores matmul contracts K=65 and applies the mask for free (the 1/sqrt(H)
    scale is folded into W_q on the host).
  - V proj (tokens stationary): V_aug tiles [tok=128, head, 65] with a ones
    column; the AV matmul's output column 64 accumulates the softmax
    denominator.
  - scores^T[j, i] per (head, j-tile, i-halfblock) -> PSUM [128, 1024];
    exp via one scalar-engine ACT per tile -> bf16 P^T in SBUF.
  - AV in the [i, d] orientation: lhsT = P^T chunk [j=128, i=128], rhs =
    V_aug [j=128, 65], accumulated over 16 j-chunks -> out[i, 65].
  - normalize: DVE reciprocal of column 64 + tensor_scalar multiply; output
    DMA in natural [token, (h d)] layout.
  - Program order: scores/exp units are the backbone; projections of the
    next head, V-projection chunks, and AV groups of the previous i-block
    are sprinkled between units so every engine stays busy and the exp
    stream starts ~10us in (DMA-paced) and never starves.
"""

import numpy as np
from ml_dtypes import bfloat16

import concourse.bass as bass
import concourse.bacc as bacc
import concourse.mybir as mybir
import concourse.tile as tile
from concourse.bass_utils import run_bass_kernel_spmd

F32 = mybir.dt.float32
BF16 = mybir.dt.bfloat16
AF = mybir.ActivationFunctionType

HID = 1024
NT = 2048
D = 64
HPC = 4  # heads per core
NCORES = 8
SCALE = float(HID) ** -0.5
KD = HID // 128  # 8 contraction chunks
NJT = NT // 128  # 16 j-tiles
IB = 1024  # i-block (exp/PSUM unit)
NIB = NT // IB  # 2
NG = IB // 128  # 8 i-groups per i-block

_CACHE = {}


def _build():
    if "nc" in _CACHE:
        return _CACHE["nc"]
    nc = bacc.Bacc("TRN2", debug=False)
    hsT_d = nc.dram_tensor("hsT", [HID, NT], BF16, kind="ExternalInput")
    wqk_d = nc.dram_tensor("wqk", [HID, HPC * 128], BF16, kind="ExternalInput")
    wv_d = nc.dram_tensor("wv", [HID, HPC * D], BF16, kind="ExternalInput")
    aux_d = nc.dram_tensor("aux", [2, HPC, NT], BF16, kind="ExternalInput")
    out_d = nc.dram_tensor("out", [NT, HPC * D], F32, kind="ExternalOutput")

    with tile.TileContext(nc) as tc:
        with tc.tile_pool(name="per", bufs=1) as per:
            hsT = per.tile([128, KD, NT], BF16, tag="hst", name="hst")
            wqk = per.tile([128, KD, HPC * 128], BF16, tag="wqk", name="wqk")
            wv = per.tile([128, KD, HPC * D], BF16, tag="wv", name="wv")
            QT = per.tile([65, HPC, NT], BF16, tag="qt", name="qt")
            KT = per.tile([65, HPC, NT], BF16, tag="kt", name="kt")
            Vau = [per.tile([128, HPC, 65], BF16, tag=f"vau{t}", name=f"vau{t}") for t in range(NJT)]

            scratch = per.tile([128, 640], BF16, tag="scr", name="scratch")
            nc.vector.memset(scratch[:], 0.0)

            hsT_r = hsT_d.ap().rearrange("(c p) m -> p c m", p=128)
            wqk_r = wqk_d.ap().rearrange("(c p) m -> p c m", p=128)
            wv_r = wv_d.ap().rearrange("(c p) m -> p c m", p=128)
            # DMA order tracks first use: head-0 W columns, first two token
            # quarters of hsT (QK proj of head 0), mask/ones rows, the rest.
            nc.sync.dma_start(out=wqk[:, :, 0:128], in_=wqk_r[:, :, 0:128])
            nc.sync.dma_start(out=hsT[:, :, 0:512], in_=hsT_r[:, :, 0:512])
            nc.sync.dma_start(out=hsT[:, :, 512:1024], in_=hsT_r[:, :, 512:1024])
            nc.sync.dma_start(out=KT[64:65, :, :], in_=aux_d.ap()[0:1])
            nc.sync.dma_start(out=QT[64:65, :, :], in_=aux_d.ap()[1:2])
            nc.sync.dma_start(out=wqk[:, :, 128:512], in_=wqk_r[:, :, 128:512])
            nc.sync.dma_start(out=hsT[:, :, NT // 2:NT], in_=hsT_r[:, :, NT // 2:NT])
            nc.sync.dma_start(out=wv[:], in_=wv_r)
            for t in range(NJT):
                nc.gpsimd.memset(Vau[t][:, :, 64:65], 1.0)

            with (
                tc.tile_pool(name="psqk", bufs=2, space="PSUM") as psqk,
                tc.tile_pool(name="pacc", bufs=2, space="PSUM") as pacc,
                tc.tile_pool(name="psc", bufs=2, space="PSUM") as psc,
                tc.tile_pool(name="ptp", bufs=3) as ptp,
                tc.tile_pool(name="stg", bufs=2) as stg,
            ):
                # PE p-state warmup: the clock ramps to full rate only after
                # ~3us of continuous matmul activity, and the input DMAs take
                # ~7us anyway — burn the ramp on scratch matmuls.
                for w in range(18):
                    wacc = psqk.tile([128, 512], F32, tag="qk", name="wacc")
                    nc.tensor.matmul(
                        wacc[:], scratch[:, 0:128], scratch[:, 128:640],
                        start=True, stop=True,
                    )
                PTs = {}  # (h, ib) -> [pt tile per jt]
                outsb = {}  # (h, ib) -> staging tile

                def qk_chunk(h, t):
                    ts = slice(t * 512, (t + 1) * 512)
                    acc = psqk.tile([128, 512], F32, tag="qk", name="acc")
                    for k in range(KD):
                        nc.tensor.matmul(
                            acc[:],
                            wqk[:, k, h * 128 : (h + 1) * 128],
                            hsT[:, k, ts],
                            start=(k == 0),
                            stop=(k == KD - 1),
                        )
                    nc.vector.tensor_copy(QT[0:64, h, ts], acc[0:64, :])
                    nc.vector.tensor_copy(KT[0:64, h, ts], acc[64:128, :])

                def v_chunk(t):
                    pv = pacc.tile([128, HPC * D], F32, tag="acc", name="pv")
                    for k in range(KD):
                        nc.tensor.matmul(
                            pv[:],
                            hsT[:, k, t * 128 : (t + 1) * 128],
                            wv[:, k, :],
                            start=(k == 0),
                            stop=(k == KD - 1),
                        )
                    nc.vector.tensor_copy(
                        Vau[t][:, :, 0:64],
                        pv[:].rearrange("p (h d) -> p h d", h=HPC),
                    )

                def score_unit(h, ib, jt):
                    i0 = ib * IB
                    sc = psc.tile([128, IB], F32, tag="sc", name="sc")
                    for ic in range(IB // 512):
                        cs = slice(ic * 512, (ic + 1) * 512)
                        qs = slice(i0 + ic * 512, i0 + (ic + 1) * 512)
                        nc.tensor.matmul(
                            sc[:, cs],
                            KT[0:65, h, jt * 128 : (jt + 1) * 128],
                            QT[0:65, h, qs],
                            start=True,
                            stop=True,
                        )
                    pt = ptp.tile([128, IB], BF16, tag=f"pt{jt}", name="pt")
                    nc.scalar.activation(pt[:], sc[:], AF.Exp)
                    PTs[(h, ib)][jt] = pt

                def av_group(h, ib, g):
                    if g == 0:
                        outsb[(h, ib)] = stg.tile(
                            [128, NG, D], F32, tag="ob", name="ob", bufs=3
                        )
                    ob = outsb[(h, ib)]
                    av = pacc.tile([128, 128], F32, tag="acc", name="av")
                    jts = [(2 * g + i) % NJT for i in range(NJT)]
                    for i, jt in enumerate(jts):
                        nc.tensor.matmul(
                            av[:, 0:65],
                            PTs[(h, ib)][jt][:, g * 128 : (g + 1) * 128],
                            Vau[jt][:, h, :],
                            start=(i == 0),
                            stop=(i == NJT - 1),
                        )
                    rl = stg.tile([128, 1], F32, tag="rl", name="rl")
                    with nc.allow_low_precision("fp32 reciprocal"):
                        nc.vector.reciprocal(rl[:], av[:, 64:65])
                    nc.vector.tensor_scalar_mul(ob[:, g, :], av[:, 0:64], rl[:])
                    if g == NG // 2 - 1 or g == NG - 1:
                        gs = slice(0, NG // 2) if g < NG // 2 else slice(NG // 2, NG)
                        nc.sync.dma_start(
                            out=out_d.ap().rearrange("(a p) m -> p a m", p=128)[
                                :,
                                ib * NG + gs.start : ib * NG + gs.stop,
                                h * D : (h + 1) * D,
                            ],
                            in_=ob[:, gs, :],
                        )
                    if g == NG - 1:
                        del PTs[(h, ib)]
                        del outsb[(h, ib)]

                # Per-head extras schedule: unit u (= ib*16+jt) -> closures
                # sprinkled after that scores unit. Placement tracks data
                # arrival (DMA halves) and ACT progress (AV after its
                # i-block's exps are done; next head's proj under this
                # head's exp stream).
                def extras_for(h):
                    # AV work of head h-1 and the projection of head h+1 ride
                    # under head h's exp stream; V-projection chunks ride
                    # under head 0's. The last head's final i-block AV is
                    # interleaved batch-wise instead (see below).
                    ex = {}
                    qk = lambda hh, tt: (lambda: qk_chunk(hh, tt))
                    av = lambda hh, ib, gg: (lambda: av_group(hh, ib, gg))
                    vp = lambda tt: (lambda: v_chunk(tt))
                    if h == 0:
                        ex[1] = [qk(1, 0)]
                        ex[3] = [qk(1, 1)]
                        ex[5] = [qk(0, 2)]
                        ex[7] = [qk(0, 3)]
                        for t in range(12):
                            ex[9 + t] = [vp(t)]
                    elif h == 1:
                        ex[0] = [qk(1, 2)]
                        ex[2] = [qk(1, 3)]
                        for t in range(4):
                            ex[4 + t] = [vp(12 + t)]
                        for g in range(NG):
                            ex[9 + g] = [av(0, 0, g)]
                        ex[18] = [qk(2, 0)]
                        ex[20] = [qk(2, 1)]
                        for g in range(NG):
                            ex[22 + g] = [av(0, 1, g)]
                    elif h < HPC - 1:
                        ex[0] = [qk(h, 2)]
                        ex[2] = [qk(h, 3)]
                        for g in range(NG):
                            ex[4 + g] = [av(h - 1, 0, g)]
                        ex[13] = [qk(h + 1, 0)]
                        ex[15] = [qk(h + 1, 1)]
                        for g in range(NG):
                            ex[17 + g] = [av(h - 1, 1, g)]
                    else:
                        ex[0] = [qk(h, 2)]
                        ex[2] = [qk(h, 3)]
                        for g in range(NG):
                            ex[4 + g] = [av(h - 1, 0, g)]
                        for g in range(NG):
                            ex[13 + g] = [av(h - 1, 1, g)]
                        for g in range(NG):
                            ex[22 + g] = [av(h, 0, g)]
                    return ex

                qk_chunk(0, 0)
                qk_chunk(0, 1)
                hl = HPC - 1
                av8 = [None, None]
                ob_last = None
                for h in range(HPC):
                    ex = extras_for(h)
                    for ib in range(NIB):
                        PTs[(h, ib)] = [None] * NJT
                        for jt in range(NJT):
                            score_unit(h, ib, jt)
                            for fn in ex.get(ib * NJT + jt, ()):
                                fn()
                            if h == hl and ib == 1:
                                # final i-block: AV batches ride 2 units
                                # behind the exp stream (all 8 i-groups
                                # accumulate in parallel in 2 psum banks)
                                if jt == 0:
                                    av8[0] = psqk.tile(
                                        [128, 512], F32, tag="qk", name="av8a"
                                    )
                                    av8[1] = psqk.tile(
                                        [128, 512], F32, tag="qk", name="av8b"
                                    )
                                    ob_last = stg.tile(
                                        [128, NG, D], F32, tag="ob", name="ob", bufs=3
                                    )
                                bjts = [jt - 2] if jt >= 2 else []
                                if jt == NJT - 1:
                                    bjts = [NJT - 3, NJT - 2, NJT - 1]
                                for bjt in bjts:
                                    for g in range(NG):
                                        q = (g % 4) * 128
                                        # start=True clears has_written for
                                        # the WHOLE bank, so only the first
                                        # group per bank may issue it; the
                                        # other groups' first matmuls
                                        # overwrite (bit unset) and then
                                        # accumulate.
                                        nc.tensor.matmul(
                                            av8[g // 4][:, q : q + 65],
                                            PTs[(hl, 1)][bjt][
                                                :, g * 128 : (g + 1) * 128
                                            ],
                                            Vau[bjt][:, hl, :],
                                            start=(bjt == 0 and g % 4 == 0),
                                            stop=(bjt == NJT - 1),
                                            skip_group_check=True,
                                        )
                # tail: normalize the final i-block (reciprocal on DVE; the
                # multiplies alternate DVE / scalar engine, both idle here)
                for g in range(NG):
                    q = (g % 4) * 128
                    rl = stg.tile([128, 1], F32, tag="rl", name="rl")
                    with nc.allow_low_precision("fp32 reciprocal"):
                        nc.vector.reciprocal(rl[:], av8[g // 4][:, q + 64 : q + 65])
                    if g % 2 == 0:
                        nc.scalar.mul(ob_last[:, g, :], av8[g // 4][:, q : q + 64], rl[:])
                    else:
                        nc.vector.tensor_scalar_mul(
                            ob_last[:, g, :], av8[g // 4][:, q : q + 64], rl[:]
                        )
                    if g == NG // 2 - 1 or g == NG - 1:
                        gs = slice(0, NG // 2) if g < NG // 2 else slice(NG // 2, NG)
                        nc.sync.dma_start(
                            out=out_d.ap().rearrange("(a p) m -> p a m", p=128)[
                                :,
                                NG + gs.start : NG + gs.stop,
                                hl * D : (hl + 1) * D,
                            ],
                            in_=ob_last[:, gs, :],
                        )
    if not nc.is_finalized():
        nc.finalize()
    _CACHE["nc"] = nc
    return nc


def kernel(hidden_states, attention_mask, W_qkv):
    hs = np.asarray(hidden_states, dtype=np.float32)  # [2, 2048, 1024]
    am = np.asarray(attention_mask)  # [2, 2048]
    W = np.asarray(W_qkv, dtype=np.float32)  # [16, 1024, 192]

    nc = _build()
    in_maps = []
    for core in range(NCORES):
        b, hg = core // 4, core % 4
        Wc = W[hg * 4 : hg * 4 + 4]  # [4, 1024, 192]
        qk_cols = []
        for h in range(HPC):
            qk_cols.append(Wc[h, :, 0:64] * SCALE)  # q, pre-scaled
            qk_cols.append(Wc[h, :, 64:128])  # k
        aux = np.empty((2, HPC, NT), np.float32)
        aux[0] = (((am[b] != 0).astype(np.float32) - 1.0) * 30000.0)[None, :]
        aux[1] = 1.0
        in_maps.append(
            {
                "hsT": np.ascontiguousarray(hs[b].T).astype(bfloat16),
                "wqk": np.concatenate(qk_cols, axis=1).astype(bfloat16),
                "wv": np.concatenate(
                    [Wc[h, :, 128:192] for h in range(HPC)], axis=1
                ).astype(bfloat16),
                "aux": aux.astype(bfloat16),
            }
        )
    res = run_bass_kernel_spmd(nc, in_maps, list(range(NCORES)))
    if res.exec_time_ns is not None:
        print(f"HW exec time: {res.exec_time_ns} ns")
    if res.mean_exec_time_ns is not None:
        print(f"HW exec time (mean across cores): {res.mean_exec_time_ns} ns")
    out = np.empty((2, NT, HID), dtype=np.float32)
    for core in range(NCORES):
        b, hg = core // 4, core % 4
        out[b, :, hg * 256 : (hg + 1) * 256] = res.results[core]["out"]
    return out


def predicted_exec_ns():
    """Device-occupancy estimate for one core (all 8 run the same program
    in parallel). Used by test.py; the real NTFF profiling hook is not
    available in this container."""
    nc = _build()
    from concourse.timeline_sim import TimelineSim
    return float(TimelineSim(nc, trace=False).simulate())


# revision 15
# speedup vs baseline: 1.6603x; 1.0184x over previous
"""MHSA Trainium2 kernel: B=2, N=2048, H=1024, 16 heads x d=64, fp32 in/out.

Sharding: 8 cores = 2 (batch) x 4 (head-groups of 4 heads). Each core is
fully independent (no collectives); host gathers per-core [2048, 256]
outputs into [2, 2048, 1024].

Per-core structure (the scalar engine's exp stream is the critical path at
~133us; PE work is ~126us and is interleaved into the exp stream's slack):
  - All matmul operands bf16 (fp32 PSUM accumulation; fp32 normalize).
  - QK proj (W stationary): QT/KT in [65, head, tok] layout. Row 64 of KT
    holds the additive mask bias per key, row 64 of QT holds ones, so the
    scores matmul contracts K=65 and applies the mask for free (the 1/sqrt(H)
    scale is folded into W_q on the host).
  - V proj (tokens stationary): V_aug tiles [tok=128, head, 65] with a ones
    column; the AV matmul's output column 64 accumulates the softmax
    denominator.
  - scores^T[j, i] per (head, j-tile, i-halfblock) -> PSUM [128, 1024];
    exp via one scalar-engine ACT per tile -> bf16 P^T in SBUF.
  - AV in the [i, d] orientation: lhsT = P^T chunk [j=128, i=128], rhs =
    V_aug [j=128, 65], accumulated over 16 j-chunks -> out[i, 65].
  - normalize: DVE reciprocal of column 64 + tensor_scalar multiply; output
    DMA in natural [token, (h d)] layout.
  - Program order: scores/exp units are the backbone; projections of the
    next head, V-projection chunks, and AV groups of the previous i-block
    are sprinkled between units so every engine stays busy and the exp
    stream starts ~10us in (DMA-paced) and never starves.
"""

import numpy as np
from ml_dtypes import bfloat16

import concourse.bass as bass
import concourse.bacc as bacc
import concourse.mybir as mybir
import concourse.tile as tile
from concourse.bass_utils import run_bass_kernel_spmd

F32 = mybir.dt.float32
BF16 = mybir.dt.bfloat16
AF = mybir.ActivationFunctionType

HID = 1024
NT = 2048
D = 64
HPC = 4  # heads per core
NCORES = 8
SCALE = float(HID) ** -0.5
KD = HID // 128  # 8 contraction chunks
NJT = NT // 128  # 16 j-tiles
IB = 1024  # i-block (exp/PSUM unit)
NIB = NT // IB  # 2
NG = IB // 128  # 8 i-groups per i-block

_CACHE = {}


def _build():
    if "nc" in _CACHE:
        return _CACHE["nc"]
    nc = bacc.Bacc("TRN2", debug=False)
    hsT_d = nc.dram_tensor("hsT", [HID, NT], BF16, kind="ExternalInput")
    wqk_d = nc.dram_tensor("wqk", [HID, HPC * 128], BF16, kind="ExternalInput")
    wv_d = nc.dram_tensor("wv", [HID, HPC * D], BF16, kind="ExternalInput")
    aux_d = nc.dram_tensor("aux", [2, HPC, NT], BF16, kind="ExternalInput")
    out_d = nc.dram_tensor("out", [NT, HPC * D], F32, kind="ExternalOutput")

    with tile.TileContext(nc) as tc:
        with tc.tile_pool(name="per", bufs=1) as per:
            hsT = per.tile([128, KD, NT], BF16, tag="hst", name="hst")
            wqk = per.tile([128, KD, HPC * 128], BF16, tag="wqk", name="wqk")
            wv = per.tile([128, KD, HPC * D], BF16, tag="wv", name="wv")
            QT = per.tile([65, HPC, NT], BF16, tag="qt", name="qt")
            KT = per.tile([65, HPC, NT], BF16, tag="kt", name="kt")
            Vau = [per.tile([128, HPC, 65], BF16, tag=f"vau{t}", name=f"vau{t}") for t in range(NJT)]

            scratch = per.tile([128, 640], BF16, tag="scr", name="scratch")
            nc.vector.memset(scratch[:], 0.0)

            hsT_r = hsT_d.ap().rearrange("(c p) m -> p c m", p=128)
            wqk_r = wqk_d.ap().rearrange("(c p) m -> p c m", p=128)
            wv_r = wv_d.ap().rearrange("(c p) m -> p c m", p=128)
            # DMA order tracks first use: head-0 W columns, first two token
            # quarters of hsT (QK proj of head 0), mask/ones rows, the rest.
            nc.sync.dma_start(out=wqk[:, :, 0:128], in_=wqk_r[:, :, 0:128])
            nc.sync.dma_start(out=hsT[:, :, 0:512], in_=hsT_r[:, :, 0:512])
            nc.sync.dma_start(out=hsT[:, :, 512:1024], in_=hsT_r[:, :, 512:1024])
            nc.sync.dma_start(out=KT[64:65, :, :], in_=aux_d.ap()[0:1])
            nc.sync.dma_start(out=QT[64:65, :, :], in_=aux_d.ap()[1:2])
            nc.sync.dma_start(out=wqk[:, :, 128:512], in_=wqk_r[:, :, 128:512])
            nc.sync.dma_start(out=hsT[:, :, NT // 2:NT], in_=hsT_r[:, :, NT // 2:NT])
            nc.sync.dma_start(out=wv[:], in_=wv_r)
            for t in range(NJT):
                nc.gpsimd.memset(Vau[t][:, :, 64:65], 1.0)

            with (
                tc.tile_pool(name="psqk", bufs=2, space="PSUM") as psqk,
                tc.tile_pool(name="pacc", bufs=2, space="PSUM") as pacc,
                tc.tile_pool(name="psc", bufs=2, space="PSUM") as psc,
                tc.tile_pool(name="ptp", bufs=3) as ptp,
                tc.tile_pool(name="stg", bufs=2) as stg,
            ):
                # PE p-state warmup: the clock ramps to full rate only after
                # ~3us of continuous matmul activity, and the input DMAs take
                # ~7us anyway — burn the ramp on scratch matmuls.
                for w in range(18):
                    wacc = psqk.tile([128, 512], F32, tag="qk", name="wacc")
                    nc.tensor.matmul(
                        wacc[:], scratch[:, 0:128], scratch[:, 128:640],
                        start=True, stop=True,
                    )
                PTs = {}  # (h, ib) -> [pt tile per jt]
                outsb = {}  # (h, ib) -> staging tile

                qk_acc = {}

                def qk_part(h, t, part):
                    # half of a projection chunk (4 of 8 contraction matmuls)
                    # so a single extras slot stays under the exp-unit pace;
                    # the accumulation group stays open across the split.
                    ts = slice(t * 512, (t + 1) * 512)
                    if part == 0:
                        qk_acc[(h, t)] = psqk.tile(
                            [128, 512], F32, tag="qk", name="acc"
                        )
                    acc = qk_acc[(h, t)]
                    for k in range(part * 4, part * 4 + 4):
                        nc.tensor.matmul(
                            acc[:],
                            wqk[:, k, h * 128 : (h + 1) * 128],
                            hsT[:, k, ts],
                            start=(k == 0),
                            stop=(k == KD - 1),
                        )
                    if part == 1:
                        nc.vector.tensor_copy(QT[0:64, h, ts], acc[0:64, :])
                        nc.vector.tensor_copy(KT[0:64, h, ts], acc[64:128, :])
                        del qk_acc[(h, t)]

                def qk_chunk(h, t):
                    qk_part(h, t, 0)
                    qk_part(h, t, 1)

                def v_chunk(t):
                    pv = pacc.tile([128, HPC * D], F32, tag="acc", name="pv")
                    for k in range(KD):
                        nc.tensor.matmul(
                            pv[:],
                            hsT[:, k, t * 128 : (t + 1) * 128],
                            wv[:, k, :],
                            start=(k == 0),
                            stop=(k == KD - 1),
                        )
                    nc.vector.tensor_copy(
                        Vau[t][:, :, 0:64],
                        pv[:].rearrange("p (h d) -> p h d", h=HPC),
                    )

                def score_unit(h, ib, jt):
                    i0 = ib * IB
                    sc = psc.tile([128, IB], F32, tag="sc", name="sc")
                    for ic in range(IB // 512):
                        cs = slice(ic * 512, (ic + 1) * 512)
                        qs = slice(i0 + ic * 512, i0 + (ic + 1) * 512)
                        nc.tensor.matmul(
                            sc[:, cs],
                            KT[0:65, h, jt * 128 : (jt + 1) * 128],
                            QT[0:65, h, qs],
                            start=True,
                            stop=True,
                        )
                    pt = ptp.tile([128, IB], BF16, tag=f"pt{jt}", name="pt")
                    nc.scalar.activation(pt[:], sc[:], AF.Exp)
                    PTs[(h, ib)][jt] = pt

                def av_group(h, ib, g):
                    if g == 0:
                        outsb[(h, ib)] = stg.tile(
                            [128, NG, D], F32, tag="ob", name="ob", bufs=3
                        )
                    ob = outsb[(h, ib)]
                    av = pacc.tile([128, 128], F32, tag="acc", name="av")
                    jts = [(2 * g + i) % NJT for i in range(NJT)]
                    for i, jt in enumerate(jts):
                        nc.tensor.matmul(
                            av[:, 0:65],
                            PTs[(h, ib)][jt][:, g * 128 : (g + 1) * 128],
                            Vau[jt][:, h, :],
                            start=(i == 0),
                            stop=(i == NJT - 1),
                        )
                    rl = stg.tile([128, 1], F32, tag="rl", name="rl")
                    with nc.allow_low_precision("fp32 reciprocal"):
                        nc.vector.reciprocal(rl[:], av[:, 64:65])
                    nc.vector.tensor_scalar_mul(ob[:, g, :], av[:, 0:64], rl[:])
                    if g == NG // 2 - 1 or g == NG - 1:
                        gs = slice(0, NG // 2) if g < NG // 2 else slice(NG // 2, NG)
                        nc.sync.dma_start(
                            out=out_d.ap().rearrange("(a p) m -> p a m", p=128)[
                                :,
                                ib * NG + gs.start : ib * NG + gs.stop,
                                h * D : (h + 1) * D,
                            ],
                            in_=ob[:, gs, :],
                        )
                    if g == NG - 1:
                        del PTs[(h, ib)]
                        del outsb[(h, ib)]

                # Per-head extras schedule: unit u (= ib*16+jt) -> closures
                # sprinkled after that scores unit. Placement tracks data
                # arrival (DMA halves) and ACT progress (AV after its
                # i-block's exps are done; next head's proj under this
                # head's exp stream).
                def extras_for(h):
                    # AV work of head h-1 and the projection of head h+1 ride
                    # under head h's exp stream; V-projection chunks ride
                    # under head 0's. The last head's final i-block AV is
                    # interleaved batch-wise instead (see below).
                    ex = {}
                    qk = lambda hh, tt, pp: (lambda: qk_part(hh, tt, pp))
                    av = lambda hh, ib, gg: (lambda: av_group(hh, ib, gg))
                    vp = lambda tt: (lambda: v_chunk(tt))

                    def put(slots, items):
                        for u, it in zip(slots, items):
                            ex.setdefault(u, []).append(it)

                    if h == 0:
                        put([1, 2], [qk(1, 0, 0), qk(1, 0, 1)])
                        put([4, 5], [qk(0, 2, 0), qk(0, 2, 1)])
                        put([7, 8], [qk(0, 3, 0), qk(0, 3, 1)])
                        put([10, 11], [qk(1, 1, 0), qk(1, 1, 1)])
                        put([13, 14, 16, 17, 19, 20, 22, 23, 25, 26, 28, 29],
                            [vp(t) for t in range(12)])
                    elif h == 1:
                        put([0, 1], [qk(1, 2, 0), qk(1, 2, 1)])
                        put([3, 4], [qk(1, 3, 0), qk(1, 3, 1)])
                        put([6, 7, 9, 10], [vp(t) for t in range(12, 16)])
                        put(range(12, 20), [av(0, 0, g) for g in range(NG)])
                        put([20, 21], [qk(2, 0, 0), qk(2, 0, 1)])
                        put([22, 23], [qk(2, 1, 0), qk(2, 1, 1)])
                        put(range(24, 32), [av(0, 1, g) for g in range(NG)])
                    elif h < HPC - 1:
                        put([0, 1], [qk(h, 2, 0), qk(h, 2, 1)])
                        put([3, 4], [qk(h, 3, 0), qk(h, 3, 1)])
                        put(range(6, 14), [av(h - 1, 0, g) for g in range(NG)])
                        put([15, 16], [qk(h + 1, 0, 0), qk(h + 1, 0, 1)])
                        put([18, 19], [qk(h + 1, 1, 0), qk(h + 1, 1, 1)])
                        put(range(21, 29), [av(h - 1, 1, g) for g in range(NG)])
                    else:
                        put([0, 1], [qk(h, 2, 0), qk(h, 2, 1)])
                        put([3, 4], [qk(h, 3, 0), qk(h, 3, 1)])
                        put(range(6, 14), [av(h - 1, 0, g) for g in range(NG)])
                        put(range(15, 23), [av(h - 1, 1, g) for g in range(NG)])
                        put(range(24, 32), [av(h, 0, g) for g in range(NG)])
                    return ex

                qk_chunk(0, 0)
                qk_chunk(0, 1)
                hl = HPC - 1
                av8 = [None, None]
                ob_last = None
                for h in range(HPC):
                    ex = extras_for(h)
                    for ib in range(NIB):
                        PTs[(h, ib)] = [None] * NJT
                        for jt in range(NJT):
                            score_unit(h, ib, jt)
                            for fn in ex.get(ib * NJT + jt, ()):
                                fn()
                            if h == hl and ib == 1:
                                # final i-block: AV batches ride 2 units
                                # behind the exp stream (all 8 i-groups
                                # accumulate in parallel in 2 psum banks)
                                if jt == 0:
                                    av8[0] = psqk.tile(
                                        [128, 512], F32, tag="qk", name="av8a"
                                    )
                                    av8[1] = psqk.tile(
                                        [128, 512], F32, tag="qk", name="av8b"
                                    )
                                    ob_last = stg.tile(
                                        [128, NG, D], F32, tag="ob", name="ob", bufs=3
                                    )
                                bjts = [jt - 2] if jt >= 2 else []
                                if jt == NJT - 1:
                                    bjts = [NJT - 3, NJT - 2, NJT - 1]
                                for bjt in bjts:
                                    for g in range(NG):
                                        q = (g % 4) * 128
                                        # start=True clears has_written for
                                        # the WHOLE bank, so only the first
                                        # group per bank may issue it; the
                                        # other groups' first matmuls
                                        # overwrite (bit unset) and then
                                        # accumulate.
                                        nc.tensor.matmul(
                                            av8[g // 4][:, q : q + 65],
                                            PTs[(hl, 1)][bjt][
                                                :, g * 128 : (g + 1) * 128
                                            ],
                                            Vau[bjt][:, hl, :],
                                            start=(bjt == 0 and g % 4 == 0),
                                            stop=(bjt == NJT - 1),
                                            skip_group_check=True,
                                        )
                # tail: normalize the final i-block (reciprocal on DVE; the
                # multiplies alternate DVE / scalar engine, both idle here)
                for g in range(NG):
                    q = (g % 4) * 128
                    rl = stg.tile([128, 1], F32, tag="rl", name="rl")
                    with nc.allow_low_precision("fp32 reciprocal"):
                        nc.vector.reciprocal(rl[:], av8[g // 4][:, q + 64 : q + 65])
                    if g % 2 == 0:
                        nc.scalar.mul(ob_last[:, g, :], av8[g // 4][:, q : q + 64], rl[:])
                    else:
                        nc.vector.tensor_scalar_mul(
                            ob_last[:, g, :], av8[g // 4][:, q : q + 64], rl[:]
                        )
                    if g == NG // 2 - 1 or g == NG - 1:
                        gs = slice(0, NG // 2) if g < NG // 2 else slice(NG // 2, NG)
                        nc.sync.dma_start(
                            out=out_d.ap().rearrange("(a p) m -> p a m", p=128)[
                                :,
                                NG + gs.start : NG + gs.stop,
                                hl * D : (hl + 1) * D,
                            ],
                            in_=ob_last[:, gs, :],
                        )
    if not nc.is_finalized():
        nc.finalize()
    _CACHE["nc"] = nc
    return nc


def kernel(hidden_states, attention_mask, W_qkv):
    hs = np.asarray(hidden_states, dtype=np.float32)  # [2, 2048, 1024]
    am = np.asarray(attention_mask)  # [2, 2048]
    W = np.asarray(W_qkv, dtype=np.float32)  # [16, 1024, 192]

    nc = _build()
    in_maps = []
    for core in range(NCORES):
        b, hg = core // 4, core % 4
        Wc = W[hg * 4 : hg * 4 + 4]  # [4, 1024, 192]
        qk_cols = []
        for h in range(HPC):
            qk_cols.append(Wc[h, :, 0:64] * SCALE)  # q, pre-scaled
            qk_cols.append(Wc[h, :, 64:128])  # k
        aux = np.empty((2, HPC, NT), np.float32)
        aux[0] = (((am[b] != 0).astype(np.float32) - 1.0) * 30000.0)[None, :]
        aux[1] = 1.0
        in_maps.append(
            {
                "hsT": np.ascontiguousarray(hs[b].T).astype(bfloat16),
                "wqk": np.concatenate(qk_cols, axis=1).astype(bfloat16),
                "wv": np.concatenate(
                    [Wc[h, :, 128:192] for h in range(HPC)], axis=1
                ).astype(bfloat16),
                "aux": aux.astype(bfloat16),
            }
        )
    res = run_bass_kernel_spmd(nc, in_maps, list(range(NCORES)))
    if res.exec_time_ns is not None:
        print(f"HW exec time: {res.exec_time_ns} ns")
    if res.mean_exec_time_ns is not None:
        print(f"HW exec time (mean across cores): {res.mean_exec_time_ns} ns")
    out = np.empty((2, NT, HID), dtype=np.float32)
    for core in range(NCORES):
        b, hg = core // 4, core % 4
        out[b, :, hg * 256 : (hg + 1) * 256] = res.results[core]["out"]
    return out


def predicted_exec_ns():
    """Device-occupancy estimate for one core (all 8 run the same program
    in parallel). Used by test.py; the real NTFF profiling hook is not
    available in this container."""
    nc = _build()
    from concourse.timeline_sim import TimelineSim
    return float(TimelineSim(nc, trace=False).simulate())


# revision 18
# speedup vs baseline: 1.6607x; 1.0003x over previous
"""MHSA Trainium2 kernel: B=2, N=2048, H=1024, 16 heads x d=64, fp32 in/out.

Sharding: 8 cores = 2 (batch) x 4 (head-groups of 4 heads). Each core is
fully independent (no collectives); host gathers per-core [2048, 256]
outputs into [2, 2048, 1024].

Per-core structure (the scalar engine's exp stream is the critical path at
~133us; PE work is ~126us and is interleaved into the exp stream's slack):
  - All matmul operands bf16 (fp32 PSUM accumulation; fp32 normalize).
  - QK proj (W stationary): QT/KT in [65, head, tok] layout. Row 64 of KT
    holds the additive mask bias per key, row 64 of QT holds ones, so the
    scores matmul contracts K=65 and applies the mask for free (the 1/sqrt(H)
    scale is folded into W_q on the host).
  - V proj (tokens stationary): V_aug tiles [tok=128, head, 65] with a ones
    column; the AV matmul's output column 64 accumulates the softmax
    denominator.
  - scores^T[j, i] per (head, j-tile, i-halfblock) -> PSUM [128, 1024];
    exp via one scalar-engine ACT per tile -> bf16 P^T in SBUF.
  - AV in the [i, d] orientation: lhsT = P^T chunk [j=128, i=128], rhs =
    V_aug [j=128, 65], accumulated over 16 j-chunks -> out[i, 65].
  - normalize: DVE reciprocal of column 64 + tensor_scalar multiply; output
    DMA in natural [token, (h d)] layout.
  - Program order: scores/exp units are the backbone; projections of the
    next head, V-projection chunks, and AV groups of the previous i-block
    are sprinkled between units so every engine stays busy and the exp
    stream starts ~10us in (DMA-paced) and never starves.
"""

import numpy as np
from ml_dtypes import bfloat16

import concourse.bass as bass
import concourse.bacc as bacc
import concourse.mybir as mybir
import concourse.tile as tile
from concourse.bass_utils import run_bass_kernel_spmd

F32 = mybir.dt.float32
BF16 = mybir.dt.bfloat16
AF = mybir.ActivationFunctionType

HID = 1024
NT = 2048
D = 64
HPC = 4  # heads per core
NCORES = 8
SCALE = float(HID) ** -0.5
KD = HID // 128  # 8 contraction chunks
NJT = NT // 128  # 16 j-tiles
IB = 1024  # i-block (exp/PSUM unit)
NIB = NT // IB  # 2
NG = IB // 128  # 8 i-groups per i-block

_CACHE = {}


def _build():
    if "nc" in _CACHE:
        return _CACHE["nc"]
    nc = bacc.Bacc("TRN2", debug=False)
    hsT_d = nc.dram_tensor("hsT", [HID, NT], BF16, kind="ExternalInput")
    wqk_d = nc.dram_tensor("wqk", [HID, HPC * 128], BF16, kind="ExternalInput")
    wv_d = nc.dram_tensor("wv", [HID, HPC * D], BF16, kind="ExternalInput")
    aux_d = nc.dram_tensor("aux", [2, HPC, NT], BF16, kind="ExternalInput")
    out_d = nc.dram_tensor("out", [NT, HPC * D], F32, kind="ExternalOutput")

    with tile.TileContext(nc) as tc:
        with tc.tile_pool(name="per", bufs=1) as per:
            hsT = per.tile([128, KD, NT], BF16, tag="hst", name="hst")
            wqk = per.tile([128, KD, HPC * 128], BF16, tag="wqk", name="wqk")
            wv = per.tile([128, KD, HPC * D], BF16, tag="wv", name="wv")
            QT = per.tile([65, HPC, NT], BF16, tag="qt", name="qt")
            KT = per.tile([65, HPC, NT], BF16, tag="kt", name="kt")
            Vau = [per.tile([128, HPC, 65], BF16, tag=f"vau{t}", name=f"vau{t}") for t in range(NJT)]

            scratch = per.tile([128, 640], BF16, tag="scr", name="scratch")
            nc.vector.memset(scratch[:], 0.0)

            hsT_r = hsT_d.ap().rearrange("(c p) m -> p c m", p=128)
            wqk_r = wqk_d.ap().rearrange("(c p) m -> p c m", p=128)
            wv_r = wv_d.ap().rearrange("(c p) m -> p c m", p=128)
            # DMA order tracks first use: head-0 W columns, first two token
            # quarters of hsT (QK proj of head 0), mask/ones rows, the rest.
            nc.sync.dma_start(out=wqk[:, :, 0:128], in_=wqk_r[:, :, 0:128])
            nc.sync.dma_start(out=hsT[:, :, 0:512], in_=hsT_r[:, :, 0:512])
            nc.sync.dma_start(out=hsT[:, :, 512:1024], in_=hsT_r[:, :, 512:1024])
            nc.sync.dma_start(out=KT[64:65, :, :], in_=aux_d.ap()[0:1])
            nc.sync.dma_start(out=QT[64:65, :, :], in_=aux_d.ap()[1:2])
            nc.sync.dma_start(out=wqk[:, :, 128:512], in_=wqk_r[:, :, 128:512])
            nc.sync.dma_start(out=hsT[:, :, NT // 2:NT], in_=hsT_r[:, :, NT // 2:NT])
            nc.sync.dma_start(out=wv[:], in_=wv_r)
            for t in range(NJT):
                nc.gpsimd.memset(Vau[t][:, :, 64:65], 1.0)

            with (
                tc.tile_pool(name="psqk", bufs=2, space="PSUM") as psqk,
                tc.tile_pool(name="pacc", bufs=2, space="PSUM") as pacc,
                tc.tile_pool(name="psc", bufs=2, space="PSUM") as psc,
                tc.tile_pool(name="ptp", bufs=3) as ptp,
                tc.tile_pool(name="stg", bufs=2) as stg,
            ):
                # PE p-state warmup: the clock ramps to full rate only after
                # ~3us of continuous matmul activity, and the input DMAs take
                # ~7us anyway — burn the ramp on scratch matmuls.
                for w in range(18):
                    wacc = psqk.tile([128, 512], F32, tag="qk", name="wacc")
                    nc.tensor.matmul(
                        wacc[:], scratch[:, 0:128], scratch[:, 128:640],
                        start=True, stop=True,
                    )
                PTs = {}  # (h, ib) -> [pt tile per jt]
                outsb = {}  # (h, ib) -> staging tile

                qk_acc = {}

                def qk_part(h, t, part):
                    # half of a projection chunk (4 of 8 contraction matmuls)
                    # so a single extras slot stays under the exp-unit pace;
                    # the accumulation group stays open across the split.
                    ts = slice(t * 512, (t + 1) * 512)
                    if part == 0:
                        qk_acc[(h, t)] = psqk.tile(
                            [128, 512], F32, tag="qk", name="acc"
                        )
                    acc = qk_acc[(h, t)]
                    for k in range(part * 4, part * 4 + 4):
                        nc.tensor.matmul(
                            acc[:],
                            wqk[:, k, h * 128 : (h + 1) * 128],
                            hsT[:, k, ts],
                            start=(k == 0),
                            stop=(k == KD - 1),
                        )
                    if part == 1:
                        nc.vector.tensor_copy(QT[0:64, h, ts], acc[0:64, :])
                        nc.vector.tensor_copy(KT[0:64, h, ts], acc[64:128, :])
                        del qk_acc[(h, t)]

                def qk_chunk(h, t):
                    qk_part(h, t, 0)
                    qk_part(h, t, 1)

                def v_chunk(t):
                    pv = pacc.tile([128, HPC * D], F32, tag="acc", name="pv")
                    for k in range(KD):
                        nc.tensor.matmul(
                            pv[:],
                            hsT[:, k, t * 128 : (t + 1) * 128],
                            wv[:, k, :],
                            start=(k == 0),
                            stop=(k == KD - 1),
                        )
                    nc.vector.tensor_copy(
                        Vau[t][:, :, 0:64],
                        pv[:].rearrange("p (h d) -> p h d", h=HPC),
                    )

                def score_unit(h, ib, jt, lo=0, hi=IB):
                    i0 = ib * IB
                    sc = psc.tile([128, hi - lo], F32, tag="sc", name="sc")
                    for ic in range(lo // 512, hi // 512):
                        cs = slice(ic * 512 - lo, (ic + 1) * 512 - lo)
                        qs = slice(i0 + ic * 512, i0 + (ic + 1) * 512)
                        nc.tensor.matmul(
                            sc[:, cs],
                            KT[0:65, h, jt * 128 : (jt + 1) * 128],
                            QT[0:65, h, qs],
                            start=True,
                            stop=True,
                        )
                    if lo == 0:
                        PTs[(h, ib)][jt] = ptp.tile(
                            [128, IB], BF16, tag=f"pt{jt}", name="pt"
                        )
                    pt = PTs[(h, ib)][jt]
                    nc.scalar.activation(pt[:, lo:hi], sc[:], AF.Exp)

                def av_group(h, ib, g):
                    if g == 0:
                        outsb[(h, ib)] = stg.tile(
                            [128, NG, D], F32, tag="ob", name="ob", bufs=3
                        )
                    ob = outsb[(h, ib)]
                    av = pacc.tile([128, 128], F32, tag="acc", name="av")
                    jts = [(2 * g + i) % NJT for i in range(NJT)]
                    for i, jt in enumerate(jts):
                        nc.tensor.matmul(
                            av[:, 0:65],
                            PTs[(h, ib)][jt][:, g * 128 : (g + 1) * 128],
                            Vau[jt][:, h, :],
                            start=(i == 0),
                            stop=(i == NJT - 1),
                        )
                    rl = stg.tile([128, 1], F32, tag="rl", name="rl")
                    with nc.allow_low_precision("fp32 reciprocal"):
                        nc.vector.reciprocal(rl[:], av[:, 64:65])
                    nc.vector.tensor_scalar_mul(ob[:, g, :], av[:, 0:64], rl[:])
                    if g == NG // 2 - 1 or g == NG - 1:
                        gs = slice(0, NG // 2) if g < NG // 2 else slice(NG // 2, NG)
                        nc.sync.dma_start(
                            out=out_d.ap().rearrange("(a p) m -> p a m", p=128)[
                                :,
                                ib * NG + gs.start : ib * NG + gs.stop,
                                h * D : (h + 1) * D,
                            ],
                            in_=ob[:, gs, :],
                        )
                    if g == NG - 1:
                        del PTs[(h, ib)]
                        del outsb[(h, ib)]

                # Per-head extras schedule: unit u (= ib*16+jt) -> closures
                # sprinkled after that scores unit. Placement tracks data
                # arrival (DMA halves) and ACT progress (AV after its
                # i-block's exps are done; next head's proj under this
                # head's exp stream).
                def extras_for(h):
                    # AV work of head h-1 and the projection of head h+1 ride
                    # under head h's exp stream; V-projection chunks ride
                    # under head 0's. The last head's final i-block AV is
                    # interleaved batch-wise instead (see below).
                    ex = {}
                    qk = lambda hh, tt, pp: (lambda: qk_part(hh, tt, pp))
                    av = lambda hh, ib, gg: (lambda: av_group(hh, ib, gg))
                    vp = lambda tt: (lambda: v_chunk(tt))

                    def put(slots, items):
                        for u, it in zip(slots, items):
                            ex.setdefault(u, []).append(it)

                    if h == 0:
                        put([1, 2], [qk(1, 0, 0), qk(1, 0, 1)])
                        put([4, 5], [qk(0, 2, 0), qk(0, 2, 1)])
                        put([7, 8], [qk(0, 3, 0), qk(0, 3, 1)])
                        put([10, 11], [qk(1, 1, 0), qk(1, 1, 1)])
                        put([13, 14, 16, 17, 19, 20, 22, 23, 25, 26, 28, 29],
                            [vp(t) for t in range(12)])
                    elif h == 1:
                        put([0, 1], [qk(1, 2, 0), qk(1, 2, 1)])
                        put([3, 4], [qk(1, 3, 0), qk(1, 3, 1)])
                        put([6, 7, 9, 10], [vp(t) for t in range(12, 16)])
                        put(range(12, 20), [av(0, 0, g) for g in range(NG)])
                        put([20, 21], [qk(2, 0, 0), qk(2, 0, 1)])
                        put([22, 23], [qk(2, 1, 0), qk(2, 1, 1)])
                        put(range(24, 32), [av(0, 1, g) for g in range(NG)])
                    elif h < HPC - 1:
                        put([0, 1], [qk(h, 2, 0), qk(h, 2, 1)])
                        put([3, 4], [qk(h, 3, 0), qk(h, 3, 1)])
                        put(range(6, 14), [av(h - 1, 0, g) for g in range(NG)])
                        put([15, 16], [qk(h + 1, 0, 0), qk(h + 1, 0, 1)])
                        put([18, 19], [qk(h + 1, 1, 0), qk(h + 1, 1, 1)])
                        put(range(21, 29), [av(h - 1, 1, g) for g in range(NG)])
                    else:
                        put([0, 1], [qk(h, 2, 0), qk(h, 2, 1)])
                        put([3, 4], [qk(h, 3, 0), qk(h, 3, 1)])
                        put(range(5, 13), [av(h - 1, 0, g) for g in range(NG)])
                        put(range(13, 21), [av(h - 1, 1, g) for g in range(NG)])
                        put(range(21, 29), [av(h, 0, g) for g in range(NG)])
                    return ex

                qk_chunk(0, 0)
                qk_chunk(0, 1)
                hl = HPC - 1
                av8 = [None, None]
                ob_last = None
                for h in range(HPC):
                    ex = extras_for(h)
                    for ib in range(NIB):
                        PTs[(h, ib)] = [None] * NJT
                        for jt in range(NJT):
                            if h == 0 and ib == 0 and jt < 4:
                                # narrow warm-up units: the first token half
                                # of hsT lands ~3us before the second, so
                                # start the exp stream on i 0:512 only
                                score_unit(0, 0, jt, 0, 512)
                                if jt == 3:
                                    for j2 in range(4):
                                        score_unit(0, 0, j2, 512, IB)
                                        for fn in ex.get(j2, ()):
                                            fn()
                                continue
                            score_unit(h, ib, jt)
                            for fn in ex.get(ib * NJT + jt, ()):
                                fn()
                            if h == hl and ib == 1:
                                # final i-block: AV batches ride 2 units
                                # behind the exp stream (all 8 i-groups
                                # accumulate in parallel in 2 psum banks)
                                if jt == 0:
                                    av8[0] = psqk.tile(
                                        [128, 512], F32, tag="qk", name="av8a"
                                    )
                                    av8[1] = psqk.tile(
                                        [128, 512], F32, tag="qk", name="av8b"
                                    )
                                    ob_last = stg.tile(
                                        [128, NG, D], F32, tag="ob", name="ob", bufs=3
                                    )
                                bjts = [jt - 2] if jt >= 2 else []
                                if jt == NJT - 1:
                                    bjts = [NJT - 3, NJT - 2, NJT - 1]
                                for bjt in bjts:
                                    for g in range(NG):
                                        q = (g % 4) * 128
                                        # start=True clears has_written for
                                        # the WHOLE bank, so only the first
                                        # group per bank may issue it; the
                                        # other groups' first matmuls
                                        # overwrite (bit unset) and then
                                        # accumulate.
                                        nc.tensor.matmul(
                                            av8[g // 4][:, q : q + 65],
                                            PTs[(hl, 1)][bjt][
                                                :, g * 128 : (g + 1) * 128
                                            ],
                                            Vau[bjt][:, hl, :],
                                            start=(bjt == 0 and g % 4 == 0),
                                            stop=(bjt == NJT - 1),
                                            skip_group_check=True,
                                        )
                # tail: normalize the final i-block (reciprocal on DVE; the
                # multiplies alternate DVE / scalar engine, both idle here)
                for g in range(NG):
                    q = (g % 4) * 128
                    rl = stg.tile([128, 1], F32, tag="rl", name="rl")
                    with nc.allow_low_precision("fp32 reciprocal"):
                        nc.vector.reciprocal(rl[:], av8[g // 4][:, q + 64 : q + 65])
                    if g % 2 == 0:
                        nc.scalar.mul(ob_last[:, g, :], av8[g // 4][:, q : q + 64], rl[:])
                    else:
                        nc.vector.tensor_scalar_mul(
                            ob_last[:, g, :], av8[g // 4][:, q : q + 64], rl[:]
                        )
                    if g == NG // 2 - 1 or g == NG - 1:
                        gs = slice(0, NG // 2) if g < NG // 2 else slice(NG // 2, NG)
                        nc.sync.dma_start(
                            out=out_d.ap().rearrange("(a p) m -> p a m", p=128)[
                                :,
                                NG + gs.start : NG + gs.stop,
                                hl * D : (hl + 1) * D,
                            ],
                            in_=ob_last[:, gs, :],
                        )
    if not nc.is_finalized():
        nc.finalize()
    _CACHE["nc"] = nc
    return nc


def kernel(hidden_states, attention_mask, W_qkv):
    hs = np.asarray(hidden_states, dtype=np.float32)  # [2, 2048, 1024]
    am = np.asarray(attention_mask)  # [2, 2048]
    W = np.asarray(W_qkv, dtype=np.float32)  # [16, 1024, 192]

    nc = _build()
    in_maps = []
    for core in range(NCORES):
        b, hg = core // 4, core % 4
        Wc = W[hg * 4 : hg * 4 + 4]  # [4, 1024, 192]
        qk_cols = []
        for h in range(HPC):
            qk_cols.append(Wc[h, :, 0:64] * SCALE)  # q, pre-scaled
            qk_cols.append(Wc[h, :, 64:128])  # k
        aux = np.empty((2, HPC, NT), np.float32)
        aux[0] = (((am[b] != 0).astype(np.float32) - 1.0) * 30000.0)[None, :]
        aux[1] = 1.0
        in_maps.append(
            {
                "hsT": np.ascontiguousarray(hs[b].T).astype(bfloat16),
                "wqk": np.concatenate(qk_cols, axis=1).astype(bfloat16),
                "wv": np.concatenate(
                    [Wc[h, :, 128:192] for h in range(HPC)], axis=1
                ).astype(bfloat16),
                "aux": aux.astype(bfloat16),
            }
        )
    res = run_bass_kernel_spmd(nc, in_maps, list(range(NCORES)))
    if res.exec_time_ns is not None:
        print(f"HW exec time: {res.exec_time_ns} ns")
    if res.mean_exec_time_ns is not None:
        print(f"HW exec time (mean across cores): {res.mean_exec_time_ns} ns")
    out = np.empty((2, NT, HID), dtype=np.float32)
    for core in range(NCORES):
        b, hg = core // 4, core % 4
        out[b, :, hg * 256 : (hg + 1) * 256] = res.results[core]["out"]
    return out


def predicted_exec_ns():
    """Device-occupancy estimate for one core (all 8 run the same program
    in parallel). Used by test.py; the real NTFF profiling hook is not
    available in this container."""
    nc = _build()
    from concourse.timeline_sim import TimelineSim
    return float(TimelineSim(nc, trace=False).simulate())
